# revision 1
# baseline (speedup 1.0000x reference)
"""Multi-head attention (batched, key-padding mask) Trainium2 Bass kernel.

Problem: nn_MultiHeadBatched
  q,k,v: [B=4, S=2048, E=1024] fp32; mask: [B, 2048] int32 (key padding)
  16 heads, head_dim 64; torch-Linear style q/k/v/out projections.

Sharding (8 cores): core c handles batch b=c//2 and head group hg=c%2
(8 heads each).  q/k/v projections are column-parallel over the head
group; out-projection is row-parallel — each core produces a partial
[E, Sq] output and the host sums the two partials per batch (+ bo).

Key structure (single NeuronCore program, SPMD over 8 cores):
  - Host compacts the KV sequence per batch to the valid (mask!=0)
    positions and pads to a multiple of 128 (SKV).  Padded positions get
    an additive -1e30 bias before exp (folded into the ScalarE exp
    activation), contributing exactly 0 — this skips ~45% of the
    attention work for Bernoulli(0.5) masks.
  - All matmuls in bf16 with fp32 PSUM accumulation.
  - Scores are computed transposed ([kv, q]); the softmax normalizer Z
    comes from an all-ones column appended to each head's V (row 64 of
    the AV accumulation), avoiding any cross-partition reduction.
  - No max-subtraction in softmax: scores/8 are ~N(0,1), far from fp32
    overflow, and max-subtraction is mathematically a no-op here.
  - Head-slot software pipeline: slot h runs QK^T+exp for head h while
    the PE also runs AV for head h-1, interleaved kv-chunk by kv-chunk
    so the PE never idles long enough for the HAM clock gate to
    re-throttle.  The V projection fills slot 0 (which has no AV yet).
"""

import os
import sys

import numpy as np

sys.path.insert(0, "/opt/trn_rl_repo")

import concourse.bass as bass
import concourse.bacc as bacc
import concourse.mybir as mybir
import concourse.tile as tile
from concourse import bass_utils

import ml_dtypes

BF16 = ml_dtypes.bfloat16

B, SQ, E = 4, 2048, 1024
H_TOT, D = 16, 64
HPC = H_TOT // 2            # heads per core (head-group split in 2)
DHC = HPC * D               # 512 projected channels per core
NE = E // 128               # contraction chunks
NDH = DHC // 128            # dh chunks per core
NTS = SQ // 512             # 512-wide q strips
NEG = -1.0e30
SCALE = D ** -0.5

N_CORES = 8

_PROGRAM_CACHE = {}
LAST_RESULTS = None


def _chunks512(n):
    out = []
    o = 0
    while o < n:
        w = min(512, n - o)
        out.append((o, w))
        o += w
    return out


def build_program(skv):
    """Build + compile the single-core SPMD Bass program for padded KV
    length `skv` (multiple of 128)."""
    if skv in _PROGRAM_CACHE:
        return _PROGRAM_CACHE[skv]

    nkv = skv // 128
    dt = mybir.dt

    nc = bacc.Bacc(
        "TRN2",
        target_bir_lowering=False,
        debug=False,
        enable_asserts=False,
        num_devices=N_CORES,
    )

    # DRAM I/O (per-core shapes)
    qT = nc.dram_tensor("qT", [E, SQ], dt.bfloat16, kind="ExternalInput").ap()
    kT = nc.dram_tensor("kT", [E, skv], dt.bfloat16, kind="ExternalInput").ap()
    vT = nc.dram_tensor("vT", [E, skv], dt.bfloat16, kind="ExternalInput").ap()
    wqT = nc.dram_tensor("wqT", [E, DHC], dt.bfloat16, kind="ExternalInput").ap()
    wkT = nc.dram_tensor("wkT", [E, DHC], dt.bfloat16, kind="ExternalInput").ap()
    wvT = nc.dram_tensor("wvT", [E, DHC], dt.bfloat16, kind="ExternalInput").ap()
    woT = nc.dram_tensor("woT", [DHC, E], dt.bfloat16, kind="ExternalInput").ap()
    mb = nc.dram_tensor("mb", [128, nkv], dt.float32, kind="ExternalInput").ap()
    outT = nc.dram_tensor("outT", [E, SQ], dt.float32, kind="ExternalOutput").ap()

    ts = bass.ts
    kvchunks = _chunks512(skv)

    with tile.TileContext(nc) as tc:
        with tc.tile_pool(name="persist", bufs=1) as pp:
            # Persistent SBUF tensors
            wq_sb = [pp.tile([128, DHC], dt.bfloat16, name=f"wq{e}", tag=f"wq{e}") for e in range(NE)]
            wk_sb = [pp.tile([128, DHC], dt.bfloat16, name=f"wk{e}", tag=f"wk{e}") for e in range(NE)]
            wv_sb = [pp.tile([128, DHC], dt.bfloat16, name=f"wv{e}", tag=f"wv{e}") for e in range(NE)]
            wo_sb = [pp.tile([128, E], dt.bfloat16, name=f"wo{c}", tag=f"wo{c}") for c in range(NDH)]
            qh_sb = [pp.tile([128, SQ], dt.bfloat16, name=f"qh{c}", tag=f"qh{c}") for c in range(NDH)]
            kh_sb = [pp.tile([128, skv], dt.bfloat16, name=f"kh{c}", tag=f"kh{c}") for c in range(NDH)]
            # V with per-head interleaved ones column: [kv, 8*(64+1)]
            va_sb = [pp.tile([128, HPC * (D + 1)], dt.bfloat16, name=f"va{j}", tag=f"va{j}") for j in range(nkv)]
            aall_sb = [pp.tile([128, SQ], dt.bfloat16, name=f"aall{c}", tag=f"aall{c}") for c in range(NDH)]
            mb_sb = pp.tile([128, nkv], dt.float32, name="mbt", tag="mbt")

            # ones columns of the augmented V (bf16 1.0)
            for j in range(nkv):
                nc.gpsimd.memset(va_sb[j][:, D::D + 1], 1.0)

            # v inputs live until the end of attention slot 0
            vip = tc.alloc_tile_pool(name="vinp", bufs=1)
            v_sb = [vip.tile([128, skv], dt.bfloat16, name=f"v{e}", tag=f"v{e}") for e in range(NE)]

            # ---------------- Q/K projections ----------------
            with (
                tc.tile_pool(name="inp", bufs=1) as ip,
                tc.tile_pool(name="qpp", bufs=4, space="PSUM") as qpp,
                tc.tile_pool(name="kpp", bufs=2, space="PSUM") as kpp,
            ):
                q_sb = [ip.tile([128, SQ], dt.bfloat16, name=f"q{e}", tag=f"q{e}") for e in range(NE)]
                k_sb = [ip.tile([128, skv], dt.bfloat16, name=f"k{e}", tag=f"k{e}") for e in range(NE)]
                # DMA order matches first-use order
                for e in range(NE):
                    nc.sync.dma_start(wq_sb[e][:], wqT[ts(e, 128), :])
                    nc.sync.dma_start(q_sb[e][:], qT[ts(e, 128), :])
                for e in range(NE):
                    nc.sync.dma_start(wk_sb[e][:], wkT[ts(e, 128), :])
                    nc.sync.dma_start(k_sb[e][:], kT[ts(e, 128), :])
                for e in range(NE):
                    nc.sync.dma_start(wv_sb[e][:], wvT[ts(e, 128), :])
                    nc.sync.dma_start(v_sb[e][:], vT[ts(e, 128), :])
                nc.sync.dma_start(mb_sb[:], mb[:])
                for c in range(NDH):
                    nc.sync.dma_start(wo_sb[c][:], woT[ts(c, 128), :])

                for c in range(NDH):
                    # Q projection: QhT[dh, t] (transposed heads)
                    qps = [qpp.tile([128, 512], dt.float32, name=f"qps{t}", tag="qps") for t in range(NTS)]
                    for e in range(NE):
                        for t in range(NTS):
                            nc.tensor.matmul(
                                qps[t][:], wq_sb[e][:, ts(c, 128)], q_sb[e][:, ts(t, 512)],
                                start=(e == 0), stop=(e == NE - 1),
                            )
                    for t in range(NTS):
                        nc.vector.tensor_copy(qh_sb[c][:, ts(t, 512)], qps[t][:])

                    # K projection for the same dh chunk
                    for (o, w) in kvchunks:
                        kps = kpp.tile([128, 512], dt.float32, name="kps", tag="kps")
                        for e in range(NE):
                            nc.tensor.matmul(
                                kps[:, 0:w], wk_sb[e][:, ts(c, 128)], k_sb[e][:, o:o + w],
                                start=(e == 0), stop=(e == NE - 1),
                            )
                        nc.vector.tensor_copy(kh_sb[c][:, o:o + w], kps[:, 0:w])

            # ---------------- attention ----------------
            with (
                tc.tile_pool(name="ppool", bufs=2) as ppool,
                tc.tile_pool(name="npool", bufs=2) as npool,
                tc.tile_pool(name="scp", bufs=1, space="PSUM") as scp,
            ):
                vpp = tc.alloc_tile_pool(name="vpp", bufs=2, space="PSUM")
                app = None
                p_prev = None
                for h in range(HPC + 1):
                    if h < HPC:
                        c, r = h // 2, h % 2
                        qh_h = qh_sb[c][r * 64:(r + 1) * 64, :]
                        kh_h = kh_sb[c][r * 64:(r + 1) * 64, :]
                        p_cur = []
                    if h > 0:
                        hp = h - 1
                        cp, rp = hp // 2, hp % 2
                        # pass p covers q strips (2p, 2p+1); Z in row 64
                        a2 = [app.tile([D + 1, 1024], dt.float32, name=f"a2_{p}", tag="aps")
                              for p in range(2)]

                    for j in range(nkv):
                        # Dependency-free filler LDWEIGHTS keep the PE array
                        # active through short ACT-bound waits (HAM stays 8/8).
                        for _ in range(2):
                            nc.tensor.ldweights(weights=wq_sb[0][:, 0:128])
                        if h < HPC:
                            # scores^T -> exp -> P[j]  [kv, q]
                            pt = ppool.tile([128, SQ], dt.bfloat16, name=f"p{j}", tag=f"p{j}")
                            p_cur.append(pt)
                            sc = scp.tile([128, SQ], dt.float32, name="sc", tag="sc")
                            for s in range(NTS):
                                nc.tensor.matmul(
                                    sc[:, ts(s, 512)],
                                    kh_h[:, ts(j, 128)],
                                    qh_h[:, ts(s, 512)],
                                    start=True, stop=True,
                                )
                            nc.scalar.activation(
                                pt[:], sc[:],
                                mybir.ActivationFunctionType.Exp,
                                bias=mb_sb[:, j:j + 1], scale=SCALE,
                            )
                        if h == 0:
                            # V projection chunk j fills slot 0's PE slack:
                            # Vh[kv, dh], written head-interleaved into va_sb
                            vps = vpp.tile([128, DHC], dt.float32, name="vps", tag="vps")
                            for e in range(NE):
                                nc.tensor.matmul(
                                    vps[:], v_sb[e][:, ts(j, 128)], wv_sb[e][:],
                                    start=(e == 0), stop=(e == NE - 1),
                                )
                            dst = va_sb[j].rearrange("p (h x) -> p h x", x=D + 1)[:, :, 0:D]
                            src = vps.rearrange("p (h x) -> p h x", x=D)
                            nc.vector.tensor_copy(dst, src)
                        if h > 0:
                            # AV for previous head, j-major accumulation
                            for p in range(2):
                                for si in range(2):
                                    nc.tensor.matmul(
                                        a2[p][:, ts(si, 512)],
                                        va_sb[j][:, hp * (D + 1):(hp + 1) * (D + 1)],
                                        p_prev[j][:, ts(2 * p + si, 512)],
                                        start=(j == 0), stop=(j == nkv - 1),
                                    )

                    if h == 0:
                        vpp.release()
                        app = tc.alloc_tile_pool(name="app", bufs=2, space="PSUM")
                    if h > 0:
                        for p in range(2):
                            # copy A (+Z row) out of PSUM right away so the
                            # banks free for the next head's AV; the slow
                            # normalization then runs entirely from SBUF.
                            au = npool.tile([D + 1, 1024], dt.float32, name="au", tag="au", bufs=4)
                            nc.vector.tensor_copy(au[:], a2[p][:])
                            for si in range(2):
                                s = 2 * p + si
                                rz = npool.tile([1, 512], dt.float32, name="rz", tag="rz", bufs=4)
                                nc.vector.reciprocal(rz[:], au[D:D + 1, ts(si, 512)])
                                rb = npool.tile([64, 512], dt.float32, name="rb", tag="rb", bufs=4)
                                nc.gpsimd.partition_broadcast(rb[:], rz[:])
                                nc.gpsimd.tensor_mul(
                                    aall_sb[cp][rp * 64:(rp + 1) * 64, ts(s, 512)],
                                    au[0:D, ts(si, 512)], rb[:],
                                )
                    p_prev = p_cur if h < HPC else None
                app.release()

            # ---------------- out projection ----------------
            with (
                tc.tile_pool(name="opool", bufs=4) as opool,
                tc.tile_pool(name="opp", bufs=4, space="PSUM") as opp,
            ):
                for eo in range(NE):
                    for t in range(NTS):
                        ops = opp.tile([128, 512], dt.float32, name="ops", tag="ops")
                        for c in range(NDH):
                            nc.tensor.matmul(
                                ops[:], wo_sb[c][:, ts(eo, 128)], aall_sb[c][:, ts(t, 512)],
                                start=(c == 0), stop=(c == NDH - 1),
                            )
                        ob = opool.tile([128, 512], dt.float32, name="ob", tag="ob")
                        nc.vector.tensor_copy(ob[:], ops[:])
                        nc.sync.dma_start(outT[ts(eo, 128), ts(t, 512)], ob[:])
            vip.release()

    nc.compile()
    _PROGRAM_CACHE[skv] = nc
    return nc


def make_in_maps(q, k, v, mask, Wq, Wk, Wv, Wo, skv):
    """Host-side shard/compact/transpose/cast. Returns per-core input dicts."""
    in_maps = []
    valid = mask != 0
    for core in range(N_CORES):
        b, hg = core // 2, core % 2
        idx = np.nonzero(valid[b])[0]
        cnt = len(idx)

        kc = np.zeros((skv, E), np.float32)
        vc = np.zeros((skv, E), np.float32)
        kc[:cnt] = k[b][idx]
        vc[:cnt] = v[b][idx]

        mbias = np.zeros((skv,), np.float32)
        mbias[cnt:] = NEG
        # [128, nkv]: column j = kv chunk j
        mb2 = np.ascontiguousarray(mbias.reshape(-1, 128).T)

        rows = slice(hg * DHC, (hg + 1) * DHC)
        in_maps.append(dict(
            qT=np.ascontiguousarray(q[b].T).astype(BF16),
            kT=np.ascontiguousarray(kc.T).astype(BF16),
            vT=np.ascontiguousarray(vc.T).astype(BF16),
            wqT=np.ascontiguousarray(Wq[rows, :].T).astype(BF16),
            wkT=np.ascontiguousarray(Wk[rows, :].T).astype(BF16),
            wvT=np.ascontiguousarray(Wv[rows, :].T).astype(BF16),
            woT=np.ascontiguousarray(Wo[:, rows].T).astype(BF16),
            mb=mb2,
        ))
    return in_maps


def _numpy_fallback(q, k, v, mask, Wq, bq, Wk, bk, Wv, bv, Wo, bo):
    out = np.zeros((B, SQ, E), np.float32)
    for b in range(B):
        qh = (q[b] @ Wq.T + bq).reshape(SQ, H_TOT, D).transpose(1, 0, 2)
        kh = (k[b] @ Wk.T + bk).reshape(-1, H_TOT, D).transpose(1, 0, 2)
        vh = (v[b] @ Wv.T + bv).reshape(-1, H_TOT, D).transpose(1, 0, 2)
        att = np.einsum("hqd,hkd->hqk", qh, kh) * SCALE
        valid = mask[b] != 0
        if not valid.any():
            out[b] = bo
            continue
        att = np.where(valid[None, None, :], att, -np.inf)
        att = att - att.max(-1, keepdims=True)
        att = np.exp(att)
        att /= att.sum(-1, keepdims=True)
        o = np.einsum("hqk,hkd->hqd", att, vh)
        o = o.transpose(1, 0, 2).reshape(SQ, E)
        out[b] = o @ Wo.T + bo
    return out


def kernel(**inputs):
    global LAST_RESULTS
    q = np.asarray(inputs["q"], np.float32)
    k = np.asarray(inputs["k"], np.float32)
    v = np.asarray(inputs["v"], np.float32)
    mask = np.asarray(inputs["mask"])
    Wq, bq = np.asarray(inputs["Wq"], np.float32), np.asarray(inputs["bq"], np.float32)
    Wk, bk = np.asarray(inputs["Wk"], np.float32), np.asarray(inputs["bk"], np.float32)
    Wv, bv = np.asarray(inputs["Wv"], np.float32), np.asarray(inputs["bv"], np.float32)
    Wo, bo = np.asarray(inputs["Wo"], np.float32), np.asarray(inputs["bo"], np.float32)

    if any(np.abs(x).max() > 0 for x in (bq, bk, bv)):
        # q/k/v biases are zero in this problem's setup; a nonzero bias
        # would need the augmented-contraction path, so fall back.
        return _numpy_fallback(q, k, v, mask, Wq, bq, Wk, bk, Wv, bv, Wo, bo)

    valid = mask != 0
    counts = valid.sum(axis=1)
    if counts.max() == 0:
        return np.broadcast_to(bo, (B, SQ, E)).astype(np.float32).copy()

    skv = int(-(-counts.max() // 128) * 128)
    nc = build_program(skv)
    in_maps = make_in_maps(q, k, v, mask, Wq, Wk, Wv, Wo, skv)

    res = bass_utils.run_bass_kernel_spmd(nc, in_maps, core_ids=list(range(N_CORES)))
    LAST_RESULTS = res

    out = np.empty((B, SQ, E), np.float32)
    for b in range(B):
        if counts[b] == 0:
            out[b] = bo
        else:
            p0 = res.results[2 * b]["outT"]
            p1 = res.results[2 * b + 1]["outT"]
            out[b] = p0.T + p1.T + bo
    return out



# revision 4
# speedup vs baseline: 1.1239x; 1.1239x over previous
"""Multi-head attention (batched, key-padding mask) Trainium2 Bass kernel.

Problem: nn_MultiHeadBatched
  q,k,v: [B=4, S=2048, E=1024] fp32; mask: [B, 2048] int32 (key padding)
  16 heads, head_dim 64; torch-Linear style q/k/v/out projections.

Sharding (8 cores): core c handles batch b=c//2 and head group hg=c%2
(8 heads each).  q/k/v projections are column-parallel over the head
group; out-projection is row-parallel — each core produces a partial
[E, Sq] output and the host sums the two partials per batch (+ bo).

Key structure (single NeuronCore program, SPMD over 8 cores):
  - Host compacts the KV sequence per batch to the valid (mask!=0)
    positions and pads to a multiple of 128 (SKV).  Padded positions get
    an additive -1e30 bias before exp (folded into the ScalarE exp
    activation), contributing exactly 0 — this skips ~45% of the
    attention work for Bernoulli(0.5) masks.
  - All matmuls in bf16 with fp32 PSUM accumulation.
  - Scores are computed transposed ([kv, q]); the softmax normalizer Z
    comes from an all-ones column appended to each head's V (row 64 of
    the AV accumulation), avoiding any cross-partition reduction.
  - No max-subtraction in softmax: scores/8 are ~N(0,1), far from fp32
    overflow, and max-subtraction is mathematically a no-op here.
  - Head-slot software pipeline: slot h runs QK^T+exp for head h while
    the PE also runs AV for head h-1, interleaved kv-chunk by kv-chunk
    so the PE never idles long enough for the HAM clock gate to
    re-throttle.  The V projection fills slot 0 (which has no AV yet).
"""

import os
import sys

import numpy as np

sys.path.insert(0, "/opt/trn_rl_repo")

import concourse.bass as bass
import concourse.bacc as bacc
import concourse.mybir as mybir
import concourse.tile as tile
from concourse import bass_utils

import ml_dtypes

BF16 = ml_dtypes.bfloat16

B, SQ, E = 4, 2048, 1024
H_TOT, D = 16, 64
HPC = H_TOT // 2            # heads per core (head-group split in 2)
DHC = HPC * D               # 512 projected channels per core
NE = E // 128               # contraction chunks
NDH = DHC // 128            # dh chunks per core
NTS = SQ // 512             # 512-wide q strips
NEG = -1.0e30
SCALE = D ** -0.5

N_CORES = 8

_PROGRAM_CACHE = {}
LAST_RESULTS = None


def _chunks512(n):
    out = []
    o = 0
    while o < n:
        w = min(512, n - o)
        out.append((o, w))
        o += w
    return out


def build_program(skv):
    """Build + compile the single-core SPMD Bass program for padded KV
    length `skv` (multiple of 128)."""
    if skv in _PROGRAM_CACHE:
        return _PROGRAM_CACHE[skv]

    nkv = skv // 128
    dt = mybir.dt

    nc = bacc.Bacc(
        "TRN2",
        target_bir_lowering=False,
        debug=False,
        enable_asserts=False,
        num_devices=N_CORES,
    )

    # DRAM I/O (per-core shapes)
    qT = nc.dram_tensor("qT", [E, SQ], dt.bfloat16, kind="ExternalInput").ap()
    kT = nc.dram_tensor("kT", [E, skv], dt.bfloat16, kind="ExternalInput").ap()
    vT = nc.dram_tensor("vT", [E, skv], dt.bfloat16, kind="ExternalInput").ap()
    wqT = nc.dram_tensor("wqT", [E, DHC], dt.bfloat16, kind="ExternalInput").ap()
    wkT = nc.dram_tensor("wkT", [E, DHC], dt.bfloat16, kind="ExternalInput").ap()
    wvT = nc.dram_tensor("wvT", [E, DHC], dt.bfloat16, kind="ExternalInput").ap()
    woT = nc.dram_tensor("woT", [DHC, E], dt.bfloat16, kind="ExternalInput").ap()
    mb = nc.dram_tensor("mb", [128, nkv], dt.float32, kind="ExternalInput").ap()
    outT = nc.dram_tensor("outT", [E, SQ], dt.float32, kind="ExternalOutput").ap()

    ts = bass.ts
    kvchunks = _chunks512(skv)

    with tile.TileContext(nc) as tc:
        with tc.tile_pool(name="persist", bufs=1) as pp:
            # Persistent SBUF tensors
            wq_sb = [pp.tile([128, DHC], dt.bfloat16, name=f"wq{e}", tag=f"wq{e}") for e in range(NE)]
            wk_sb = [pp.tile([128, DHC], dt.bfloat16, name=f"wk{e}", tag=f"wk{e}") for e in range(NE)]
            wv_sb = [pp.tile([128, DHC], dt.bfloat16, name=f"wv{e}", tag=f"wv{e}") for e in range(NE)]
            wo_sb = [pp.tile([128, E], dt.bfloat16, name=f"wo{c}", tag=f"wo{c}") for c in range(NDH)]
            qh_sb = [pp.tile([128, SQ], dt.bfloat16, name=f"qh{c}", tag=f"qh{c}") for c in range(NDH)]
            kh_sb = [pp.tile([128, skv], dt.bfloat16, name=f"kh{c}", tag=f"kh{c}") for c in range(NDH)]
            # V with per-head interleaved ones column: [kv, 8*(64+1)]
            va_sb = [pp.tile([128, HPC * (D + 1)], dt.bfloat16, name=f"va{j}", tag=f"va{j}") for j in range(nkv)]
            aall_sb = [pp.tile([128, SQ], dt.bfloat16, name=f"aall{c}", tag=f"aall{c}") for c in range(NDH)]
            mb_sb = pp.tile([128, nkv], dt.float32, name="mbt", tag="mbt")

            # ones columns of the augmented V (bf16 1.0)
            for j in range(nkv):
                nc.gpsimd.memset(va_sb[j][:, D::D + 1], 1.0)

            # v inputs live until the end of attention slot 0
            vip = tc.alloc_tile_pool(name="vinp", bufs=1)
            v_sb = [vip.tile([128, skv], dt.bfloat16, name=f"v{e}", tag=f"v{e}") for e in range(NE)]

            # ---------------- Q/K projections ----------------
            with (
                tc.tile_pool(name="inp", bufs=1) as ip,
                tc.tile_pool(name="qpp", bufs=4, space="PSUM") as qpp,
                tc.tile_pool(name="kpp", bufs=2, space="PSUM") as kpp,
            ):
                q_sb = [ip.tile([128, SQ], dt.bfloat16, name=f"q{e}", tag=f"q{e}") for e in range(NE)]
                k_sb = [ip.tile([128, skv], dt.bfloat16, name=f"k{e}", tag=f"k{e}") for e in range(NE)]
                # DMA order matches first-use order
                for e in range(NE):
                    nc.sync.dma_start(wq_sb[e][:], wqT[ts(e, 128), :])
                    nc.sync.dma_start(q_sb[e][:], qT[ts(e, 128), :])
                for e in range(NE):
                    nc.sync.dma_start(wk_sb[e][:], wkT[ts(e, 128), :])
                    nc.sync.dma_start(k_sb[e][:], kT[ts(e, 128), :])
                for e in range(NE):
                    nc.sync.dma_start(wv_sb[e][:], wvT[ts(e, 128), :])
                    nc.sync.dma_start(v_sb[e][:], vT[ts(e, 128), :])
                nc.sync.dma_start(mb_sb[:], mb[:])
                for c in range(NDH):
                    nc.sync.dma_start(wo_sb[c][:], woT[ts(c, 128), :])

                for c in range(NDH):
                    # Q projection: QhT[dh, t] (transposed heads)
                    qps = [qpp.tile([128, 512], dt.float32, name=f"qps{t}", tag="qps") for t in range(NTS)]
                    for e in range(NE):
                        for t in range(NTS):
                            nc.tensor.matmul(
                                qps[t][:], wq_sb[e][:, ts(c, 128)], q_sb[e][:, ts(t, 512)],
                                start=(e == 0), stop=(e == NE - 1),
                            )
                    for t in range(NTS):
                        nc.vector.tensor_copy(qh_sb[c][:, ts(t, 512)], qps[t][:])

                    # K projection for the same dh chunk
                    for (o, w) in kvchunks:
                        kps = kpp.tile([128, 512], dt.float32, name="kps", tag="kps")
                        for e in range(NE):
                            nc.tensor.matmul(
                                kps[:, 0:w], wk_sb[e][:, ts(c, 128)], k_sb[e][:, o:o + w],
                                start=(e == 0), stop=(e == NE - 1),
                            )
                        nc.vector.tensor_copy(kh_sb[c][:, o:o + w], kps[:, 0:w])

            # ---------------- attention ----------------
            with (
                tc.tile_pool(name="ppool", bufs=2) as ppool,
                tc.tile_pool(name="npool", bufs=2) as npool,
                tc.tile_pool(name="scp", bufs=1, space="PSUM") as scp,
            ):
                vpp = tc.alloc_tile_pool(name="vpp", bufs=2, space="PSUM")
                app = None
                p_prev = None
                for h in range(HPC + 1):
                    if h < HPC:
                        c, r = h // 2, h % 2
                        qh_h = qh_sb[c][r * 64:(r + 1) * 64, :]
                        kh_h = kh_sb[c][r * 64:(r + 1) * 64, :]
                        p_cur = []
                    if h > 0:
                        hp = h - 1
                        cp, rp = hp // 2, hp % 2
                        # pass p covers q strips (2p, 2p+1); Z in row 64
                        a2 = [app.tile([D + 1, 1024], dt.float32, name=f"a2_{p}", tag="aps")
                              for p in range(2)]

                    for j in range(nkv):
                        # Dependency-free filler LDWEIGHTS keep the PE array
                        # active through short ACT-bound waits (HAM stays 8/8).
                        for _ in range(2):
                            nc.tensor.ldweights(weights=wq_sb[0][:, 0:128])
                        if h < HPC:
                            # scores^T -> exp -> P[j]  [kv, q], in two q-halves
                            # so exp(half 0) overlaps scores(half 1) and the
                            # next chunk's scores overlap exp(half 1).
                            pt = ppool.tile([128, SQ], dt.bfloat16, name=f"p{j}", tag=f"p{j}")
                            p_cur.append(pt)
                            for half in range(2):
                                sc = scp.tile([128, 1024], dt.float32, name=f"sc{half}", tag=f"sc{half}")
                                for s in range(2):
                                    nc.tensor.matmul(
                                        sc[:, ts(s, 512)],
                                        kh_h[:, ts(j, 128)],
                                        qh_h[:, ts(2 * half + s, 512)],
                                        start=True, stop=True,
                                    )
                                nc.scalar.activation(
                                    pt[:, ts(half, 1024)], sc[:],
                                    mybir.ActivationFunctionType.Exp,
                                    bias=mb_sb[:, j:j + 1], scale=SCALE,
                                )
                        if h == 0:
                            # V projection chunk j fills slot 0's PE slack:
                            # Vh[kv, dh], written head-interleaved into va_sb
                            vps = vpp.tile([128, DHC], dt.float32, name="vps", tag="vps")
                            for e in range(NE):
                                nc.tensor.matmul(
                                    vps[:], v_sb[e][:, ts(j, 128)], wv_sb[e][:],
                                    start=(e == 0), stop=(e == NE - 1),
                                )
                            dst = va_sb[j].rearrange("p (h x) -> p h x", x=D + 1)[:, :, 0:D]
                            src = vps.rearrange("p (h x) -> p h x", x=D)
                            nc.vector.tensor_copy(dst, src)
                        if h > 0:
                            # AV for previous head, j-major accumulation
                            for p in range(2):
                                for si in range(2):
                                    nc.tensor.matmul(
                                        a2[p][:, ts(si, 512)],
                                        va_sb[j][:, hp * (D + 1):(hp + 1) * (D + 1)],
                                        p_prev[j][:, ts(2 * p + si, 512)],
                                        start=(j == 0), stop=(j == nkv - 1),
                                    )

                    if h == 0:
                        vpp.release()
                        app = tc.alloc_tile_pool(name="app", bufs=2, space="PSUM")
                    if h > 0:
                        for p in range(2):
                            # copy A (+Z row) out of PSUM right away so the
                            # banks free for the next head's AV; the slow
                            # normalization then runs entirely from SBUF.
                            au = npool.tile([D + 1, 1024], dt.float32, name="au", tag="au", bufs=4)
                            nc.vector.tensor_copy(au[:], a2[p][:])
                            for si in range(2):
                                s = 2 * p + si
                                rz = npool.tile([1, 512], dt.float32, name="rz", tag="rz", bufs=4)
                                nc.vector.reciprocal(rz[:], au[D:D + 1, ts(si, 512)])
                                rb = npool.tile([64, 512], dt.float32, name="rb", tag="rb", bufs=4)
                                nc.gpsimd.partition_broadcast(rb[:], rz[:])
                                nc.gpsimd.tensor_mul(
                                    aall_sb[cp][rp * 64:(rp + 1) * 64, ts(s, 512)],
                                    au[0:D, ts(si, 512)], rb[:],
                                )
                    p_prev = p_cur if h < HPC else None
                app.release()

            # ---------------- out projection ----------------
            with (
                tc.tile_pool(name="opool", bufs=4) as opool,
                tc.tile_pool(name="opp", bufs=4, space="PSUM") as opp,
            ):
                for eo in range(NE):
                    for t in range(NTS):
                        ops = opp.tile([128, 512], dt.float32, name="ops", tag="ops")
                        for c in range(NDH):
                            nc.tensor.matmul(
                                ops[:], wo_sb[c][:, ts(eo, 128)], aall_sb[c][:, ts(t, 512)],
                                start=(c == 0), stop=(c == NDH - 1),
                            )
                        ob = opool.tile([128, 512], dt.float32, name="ob", tag="ob")
                        nc.vector.tensor_copy(ob[:], ops[:])
                        nc.sync.dma_start(outT[ts(eo, 128), ts(t, 512)], ob[:])
            vip.release()

    nc.compile()
    _PROGRAM_CACHE[skv] = nc
    return nc


def make_in_maps(q, k, v, mask, Wq, Wk, Wv, Wo, skv):
    """Host-side shard/compact/transpose/cast. Returns per-core input dicts."""
    in_maps = []
    valid = mask != 0
    for core in range(N_CORES):
        b, hg = core // 2, core % 2
        idx = np.nonzero(valid[b])[0]
        cnt = len(idx)

        kc = np.zeros((skv, E), np.float32)
        vc = np.zeros((skv, E), np.float32)
        kc[:cnt] = k[b][idx]
        vc[:cnt] = v[b][idx]

        mbias = np.zeros((skv,), np.float32)
        mbias[cnt:] = NEG
        # [128, nkv]: column j = kv chunk j
        mb2 = np.ascontiguousarray(mbias.reshape(-1, 128).T)

        rows = slice(hg * DHC, (hg + 1) * DHC)
        in_maps.append(dict(
            qT=np.ascontiguousarray(q[b].T).astype(BF16),
            kT=np.ascontiguousarray(kc.T).astype(BF16),
            vT=np.ascontiguousarray(vc.T).astype(BF16),
            wqT=np.ascontiguousarray(Wq[rows, :].T).astype(BF16),
            wkT=np.ascontiguousarray(Wk[rows, :].T).astype(BF16),
            wvT=np.ascontiguousarray(Wv[rows, :].T).astype(BF16),
            woT=np.ascontiguousarray(Wo[:, rows].T).astype(BF16),
            mb=mb2,
        ))
    return in_maps


def _numpy_fallback(q, k, v, mask, Wq, bq, Wk, bk, Wv, bv, Wo, bo):
    out = np.zeros((B, SQ, E), np.float32)
    for b in range(B):
        qh = (q[b] @ Wq.T + bq).reshape(SQ, H_TOT, D).transpose(1, 0, 2)
        kh = (k[b] @ Wk.T + bk).reshape(-1, H_TOT, D).transpose(1, 0, 2)
        vh = (v[b] @ Wv.T + bv).reshape(-1, H_TOT, D).transpose(1, 0, 2)
        att = np.einsum("hqd,hkd->hqk", qh, kh) * SCALE
        valid = mask[b] != 0
        if not valid.any():
            out[b] = bo
            continue
        att = np.where(valid[None, None, :], att, -np.inf)
        att = att - att.max(-1, keepdims=True)
        att = np.exp(att)
        att /= att.sum(-1, keepdims=True)
        o = np.einsum("hqk,hkd->hqd", att, vh)
        o = o.transpose(1, 0, 2).reshape(SQ, E)
        out[b] = o @ Wo.T + bo
    return out


def kernel(**inputs):
    global LAST_RESULTS
    q = np.asarray(inputs["q"], np.float32)
    k = np.asarray(inputs["k"], np.float32)
    v = np.asarray(inputs["v"], np.float32)
    mask = np.asarray(inputs["mask"])
    Wq, bq = np.asarray(inputs["Wq"], np.float32), np.asarray(inputs["bq"], np.float32)
    Wk, bk = np.asarray(inputs["Wk"], np.float32), np.asarray(inputs["bk"], np.float32)
    Wv, bv = np.asarray(inputs["Wv"], np.float32), np.asarray(inputs["bv"], np.float32)
    Wo, bo = np.asarray(inputs["Wo"], np.float32), np.asarray(inputs["bo"], np.float32)

    if any(np.abs(x).max() > 0 for x in (bq, bk, bv)):
        # q/k/v biases are zero in this problem's setup; a nonzero bias
        # would need the augmented-contraction path, so fall back.
        return _numpy_fallback(q, k, v, mask, Wq, bq, Wk, bk, Wv, bv, Wo, bo)

    valid = mask != 0
    counts = valid.sum(axis=1)
    if counts.max() == 0:
        return np.broadcast_to(bo, (B, SQ, E)).astype(np.float32).copy()

    skv = int(-(-counts.max() // 128) * 128)
    nc = build_program(skv)
    in_maps = make_in_maps(q, k, v, mask, Wq, Wk, Wv, Wo, skv)

    res = bass_utils.run_bass_kernel_spmd(nc, in_maps, core_ids=list(range(N_CORES)))
    LAST_RESULTS = res

    out = np.empty((B, SQ, E), np.float32)
    for b in range(B):
        if counts[b] == 0:
            out[b] = bo
        else:
            p0 = res.results[2 * b]["outT"]
            p1 = res.results[2 * b + 1]["outT"]
            out[b] = p0.T + p1.T + bo
    return out



# revision 10
# speedup vs baseline: 1.4946x; 1.3298x over previous
"""Multi-head attention (batched, key-padding mask) Trainium2 Bass kernel — v2.

Problem: nn_MultiHeadBatched
  q,k,v: [B=4, S=2048, E=1024] fp32; mask: [B, 2048] int32 (key padding)
  16 heads, head_dim 64; torch-Linear style q/k/v/out projections.

Sharding (8 cores): core c handles batch b=c//2 and head group hg=c%2
(8 heads each).  q/k/v projections are column-parallel over the head
group; out-projection is row-parallel — each core produces a partial
[E, Sq] output and the host sums the two partials per batch (+ bo).

v2 structure (single NeuronCore program, SPMD over 8 cores):
  - Host compacts the KV sequence per batch to the valid (mask!=0)
    positions, pads to a multiple of 128 (SKV); padded positions get an
    additive -1e30 bias folded into the ScalarE exp activation.
  - All matmuls bf16 with fp32 PSUM accumulation.
  - Scores computed transposed ([kv, q]); softmax normalizer Z comes
    from an all-ones column appended to each head's V (row 64 of the AV
    accumulation).  No max-subtraction (scores/8 ~ N(0,1)).
  - Slot pipeline over (q-half, head-pair): slot s = (half s//4, pair
    s%4) computes scores+exp for its half/pair while the PE also runs
    AV for slot s-1, interleaved chunk by chunk.  Head pairs live in
    PE row groups 0-63 / 64-127, so the two heads' score matmuls are
    issued back-to-back with tile_position (0,0)/(64,0) and execute
    CONCURRENTLY in the PE array (row tiling) — 2x score throughput.
  - The attention phase is ScalarE(exp)-bound; all projections (Q, K,
    V, out) are emitted as filler units inside the slots' chunk loops
    so the PE does them in exp-wait slack.  Q input is loaded and
    projected per q-half to cut SBUF pressure.
  - 1/Z: the Z row [1, 1024] is spread across 128 partitions via a
    SBUF->SBUF DMA, reciprocal'd as [128, 8] (DVE reciprocal is ~8
    cycles/elem — free-dim size is what costs), and DMA'd back.
"""

import os
import sys

import numpy as np

sys.path.insert(0, "/opt/trn_rl_repo")

import concourse.bass as bass
import concourse.bacc as bacc
import concourse.mybir as mybir
import concourse.tile as tile
from concourse import bass_utils

import ml_dtypes

BF16 = ml_dtypes.bfloat16

B, SQ, E = 4, 2048, 1024
H_TOT, D = 16, 64
HPC = H_TOT // 2            # heads per core (head-group split in 2)
DHC = HPC * D               # 512 projected channels per core
NE = E // 128               # contraction chunks
NDH = DHC // 128            # dh chunks per core
SQH = SQ // 2               # q-half width
NEG = -1.0e30
SCALE = D ** -0.5

N_CORES = 8

_PROGRAM_CACHE = {}
LAST_RESULTS = None


def _chunks512(n):
    out = []
    o = 0
    while o < n:
        w = min(512, n - o)
        out.append((o, w))
        o += w
    return out


def build_program(skv):
    """Build + compile the single-core SPMD Bass program for padded KV
    length `skv` (multiple of 128)."""
    if skv in _PROGRAM_CACHE:
        return _PROGRAM_CACHE[skv]

    nkv = skv // 128
    dt = mybir.dt

    nc = bacc.Bacc(
        "TRN2",
        target_bir_lowering=False,
        debug=False,
        enable_asserts=False,
        num_devices=N_CORES,
    )

    # DRAM I/O (per-core shapes)
    qT = nc.dram_tensor("qT", [E, SQ], dt.bfloat16, kind="ExternalInput").ap()
    kT = nc.dram_tensor("kT", [E, skv], dt.bfloat16, kind="ExternalInput").ap()
    vT = nc.dram_tensor("vT", [E, skv], dt.bfloat16, kind="ExternalInput").ap()
    wqT = nc.dram_tensor("wqT", [E, DHC], dt.bfloat16, kind="ExternalInput").ap()
    wkT = nc.dram_tensor("wkT", [E, DHC], dt.bfloat16, kind="ExternalInput").ap()
    wvT = nc.dram_tensor("wvT", [E, DHC], dt.bfloat16, kind="ExternalInput").ap()
    woT = nc.dram_tensor("woT", [DHC, E], dt.bfloat16, kind="ExternalInput").ap()
    mb = nc.dram_tensor("mb", [128, nkv], dt.float32, kind="ExternalInput").ap()
    outT = nc.dram_tensor("outT", [E, SQ], dt.float32, kind="ExternalOutput").ap()

    ts = bass.ts
    kvchunks = _chunks512(skv)

    with tile.TileContext(nc) as tc:
        pp = tc.alloc_tile_pool(name="persist", bufs=1)

        # Persistent SBUF tensors
        wq_sb = [pp.tile([128, DHC], dt.bfloat16, name=f"wq{e}", tag=f"wq{e}") for e in range(NE)]
        wk_sb = [pp.tile([128, DHC], dt.bfloat16, name=f"wk{e}", tag=f"wk{e}") for e in range(NE)]
        wv_sb = [pp.tile([128, DHC], dt.bfloat16, name=f"wv{e}", tag=f"wv{e}") for e in range(NE)]
        wo_sb = [pp.tile([128, E], dt.bfloat16, name=f"wo{c}", tag=f"wo{c}") for c in range(NDH)]
        qh_sb = [pp.tile([128, SQ], dt.bfloat16, name=f"qh{c}", tag=f"qh{c}") for c in range(NDH)]
        kh_sb = [pp.tile([128, skv], dt.bfloat16, name=f"kh{c}", tag=f"kh{c}") for c in range(NDH)]
        # V with per-head interleaved ones column: [kv, 8*(64+1)]
        va_sb = [pp.tile([128, HPC * (D + 1)], dt.bfloat16, name=f"va{j}", tag=f"va{j}") for j in range(nkv)]
        aall_sb = [pp.tile([128, SQ], dt.bfloat16, name=f"aall{c}", tag=f"aall{c}") for c in range(NDH)]
        mb_sb = pp.tile([128, nkv], dt.float32, name="mbt", tag="mbt")

        for j in range(nkv):
            nc.gpsimd.memset(va_sb[j][:, D::D + 1], 1.0)

        # Input pools (released as the projections complete; right-side
        # stack so mid-stream release doesn't violate LIFO pool order)
        qip = tc.alloc_tile_pool(name="qinp", bufs=1, side="right")
        kip = tc.alloc_tile_pool(name="kinp", bufs=1, side="right")
        vip = tc.alloc_tile_pool(name="vinp", bufs=1, side="right")

        # P tiles: per slot, 2 heads x nkv chunks of [128, SQH] bf16
        ppool = tc.alloc_tile_pool(name="ppool", bufs=2)

        # PSUM pools: prj 2 banks + scores 4 banks + AV 2 banks = 8
        prj = tc.alloc_tile_pool(name="prj", bufs=2, space="PSUM")
        scp = tc.alloc_tile_pool(name="scp", bufs=1, space="PSUM")
        avp = tc.alloc_tile_pool(name="avp", bufs=1, space="PSUM")

        npool = [None]  # allocated after vip release
        opool = [None]  # allocated after qip/kip release

        # ---------------- input DMAs ----------------
        q1_sb = [qip.tile([128, SQH], dt.bfloat16, name=f"q{e}", tag=f"q{e}") for e in range(NE)]
        k_sb = [kip.tile([128, skv], dt.bfloat16, name=f"k{e}", tag=f"k{e}") for e in range(NE)]
        v_sb = [vip.tile([128, skv], dt.bfloat16, name=f"v{e}", tag=f"v{e}") for e in range(NE)]
        for e in range(NE):
            nc.sync.dma_start(wq_sb[e][:], wqT[ts(e, 128), :])
            nc.sync.dma_start(q1_sb[e][:], qT[ts(e, 128), 0:SQH])
        for e in range(NE):
            nc.sync.dma_start(wk_sb[e][:], wkT[ts(e, 128), :])
            nc.sync.dma_start(k_sb[e][:], kT[ts(e, 128), :])
        nc.sync.dma_start(mb_sb[:], mb[:])
        for e in range(NE):
            nc.sync.dma_start(wv_sb[e][:], wvT[ts(e, 128), :])
            nc.sync.dma_start(v_sb[e][:], vT[ts(e, 128), :])
        for c in range(NDH):
            nc.sync.dma_start(wo_sb[c][:], woT[ts(c, 128), :])

        # ---------------- projection / out-proj unit emitters ----------------
        def kproj_unit(c, o, w):
            def emit():
                kps = prj.tile([128, 512], dt.float32, name="kps", tag="prj")
                for e in range(NE):
                    nc.tensor.matmul(
                        kps[:, 0:w], wk_sb[e][:, ts(c, 128)], k_sb[e][:, o:o + w],
                        start=(e == 0), stop=(e == NE - 1),
                    )
                nc.vector.tensor_copy(kh_sb[c][:, o:o + w], kps[:, 0:w])
            return emit

        def qproj_unit(c, half, t, q_tiles):
            def emit():
                qps = prj.tile([128, 512], dt.float32, name="qps", tag="prj")
                for e in range(NE):
                    nc.tensor.matmul(
                        qps[:], wq_sb[e][:, ts(c, 128)], q_tiles[e][:, ts(t, 512)],
                        start=(e == 0), stop=(e == NE - 1),
                    )
                nc.vector.tensor_copy(qh_sb[c][:, half * SQH + t * 512:half * SQH + (t + 1) * 512], qps[:])
            return emit

        def vproj_unit(j):
            def emit():
                vps = prj.tile([128, DHC], dt.float32, name="vps", tag="prj")
                for e in range(NE):
                    nc.tensor.matmul(
                        vps[:], v_sb[e][:, ts(j, 128)], wv_sb[e][:],
                        start=(e == 0), stop=(e == NE - 1),
                    )
                dst = va_sb[j].rearrange("p (h x) -> p h x", x=D + 1)[:, :, 0:D]
                src = vps.rearrange("p (h x) -> p h x", x=D)
                nc.vector.tensor_copy(dst, src)
            return emit

        q2_holder = {}

        def qdma2_unit():
            def emit():
                q2 = [qip.tile([128, SQH], dt.bfloat16, name=f"q{e}", tag=f"q{e}") for e in range(NE)]
                for e in range(NE):
                    nc.sync.dma_start(q2[e][:], qT[ts(e, 128), SQH:SQ])
                q2_holder["t"] = q2
            return emit

        def qproj2_unit(c, t):
            def emit():
                qps = prj.tile([128, 512], dt.float32, name="qps", tag="prj")
                for e in range(NE):
                    nc.tensor.matmul(
                        qps[:], wq_sb[e][:, ts(c, 128)], q2_holder["t"][e][:, ts(t, 512)],
                        start=(e == 0), stop=(e == NE - 1),
                    )
                nc.vector.tensor_copy(qh_sb[c][:, SQH + t * 512:SQH + (t + 1) * 512], qps[:])
            return emit

        def oproj_unit(half, eo, t):
            def emit():
                ops = prj.tile([128, 512], dt.float32, name="ops", tag="prj")
                for c in range(NDH):
                    nc.tensor.matmul(
                        ops[:], wo_sb[c][:, ts(eo, 128)],
                        aall_sb[c][:, half * SQH + t * 512:half * SQH + (t + 1) * 512],
                        start=(c == 0), stop=(c == NDH - 1),
                    )
                ob = opool[0].tile([128, 512], dt.float32, name="ob", tag="ob", bufs=4)
                nc.vector.tensor_copy(ob[:], ops[:])
                nc.sync.dma_start(
                    outT[ts(eo, 128), half * SQH + t * 512:half * SQH + (t + 1) * 512], ob[:])
            return emit

        # ---------------- normalization ----------------
        ZW = SQH // 128  # 8

        def emit_norm(hp, half, a2):
            c, r = hp // 2, hp % 2
            np_ = npool[0]
            au = np_.tile([D + 1, SQH], dt.float32, name="au", tag="au", bufs=2)
            nc.vector.tensor_copy(au[:], a2[:])
            # spread Z across partitions; reciprocal cost is free-dim-size bound
            zt = np_.tile([128, ZW], dt.float32, name="zt", tag="zt", bufs=2)
            nc.sync.dma_start(zt[:], au[D:D + 1, :])
            rz8 = np_.tile([128, ZW], dt.float32, name="rz8", tag="rz8", bufs=2)
            nc.vector.reciprocal(rz8[:], zt[:])
            rzr = np_.tile([1, SQH], dt.float32, name="rzr", tag="rzr", bufs=1)
            nc.sync.dma_start(rzr[:], rz8[:])
            rb = np_.tile([D, SQH], dt.float32, name="rb", tag="rb", bufs=1)
            nc.gpsimd.partition_broadcast(rb[:], rzr[:])
            nc.gpsimd.tensor_mul(
                aall_sb[c][r * 64:(r + 1) * 64, half * SQH:(half + 1) * SQH],
                au[0:D, :], rb[:],
            )

        # ---------------- AV machinery ----------------
        def av_step(state, k, a2h):
            ptA, ptB, pair, half = state
            local = 0 if k < nkv else 1
            j = k - nkv * local
            pts = ptA if local == 0 else ptB
            hp = 2 * pair + local
            if j == 0:
                a2h[0] = avp.tile([D + 1, SQH], dt.float32, name="a2", tag="a2")
            a2 = a2h[0]
            for t in range(2):
                nc.tensor.matmul(
                    a2[:, ts(t, 512)],
                    va_sb[j][:, hp * (D + 1):(hp + 1) * (D + 1)],
                    pts[j][:, ts(t, 512)],
                    start=(j == 0), stop=(j == nkv - 1),
                )
            if j == nkv - 1:
                emit_norm(hp, half, a2)

        # ---------------- filler schedule ----------------
        fillers = {s: [] for s in range(9)}
        fillers[0] = [vproj_unit(j) for j in range(nkv)]
        for (o, w) in kvchunks:
            fillers[0].append(kproj_unit(1, o, w))
        fillers[0] += [qproj_unit(1, 0, t, q1_sb) for t in range(2)]
        for (o, w) in kvchunks:
            fillers[1].append(kproj_unit(2, o, w))
        fillers[1] += [qproj_unit(2, 0, t, q1_sb) for t in range(2)]
        for (o, w) in kvchunks:
            fillers[2].append(kproj_unit(3, o, w))
        fillers[2] += [qproj_unit(3, 0, t, q1_sb) for t in range(2)]
        fillers[2] += [qdma2_unit()]
        fillers[2] += [qproj2_unit(0, t) for t in range(2)]
        fillers[2] += [qproj2_unit(1, t) for t in range(2)]
        fillers[3] += [qproj2_unit(2, t) for t in range(2)]
        fillers[3] += [qproj2_unit(3, t) for t in range(2)]
        fillers[5] = [oproj_unit(0, eo, t) for eo in range(4) for t in range(2)]
        fillers[6] = [oproj_unit(0, eo, t) for eo in range(4, NE) for t in range(2)]
        fillers[8] = [oproj_unit(1, eo, t) for eo in range(NE) for t in range(2)]

        # ---------------- lead-in: K(0), Q(0, H0) ----------------
        for (o, w) in kvchunks:
            kproj_unit(0, o, w)()
        for t in range(2):
            qproj_unit(0, 0, t, q1_sb)()

        # ---------------- slot loop ----------------
        prev_state = None
        for s in range(9):
            if s == 1:
                vip.release()
                npool[0] = tc.alloc_tile_pool(name="norm", bufs=1)
            if s == 4:
                kip.release()
                qip.release()
                opool[0] = tc.alloc_tile_pool(name="outp", bufs=1)

            fl = list(fillers[s])
            n_emitted = 0
            a2h = [None]

            if s < 8:
                half, pair = s // 4, s % 4
                # pA single-buffered: the consuming AV step 2j of the next
                # slot runs at chunk j//2 <= j, before exp(j) needs the buf.
                # pB double-buffered: its AV steps run in the slot's 2nd half.
                ptA = [ppool.tile([128, SQH], dt.bfloat16, name=f"pA{j}", tag=f"pA{j}", bufs=1) for j in range(nkv)]
                ptB = [ppool.tile([128, SQH], dt.bfloat16, name=f"pB{j}", tag=f"pB{j}", bufs=2) for j in range(nkv)]
                cur_state = (ptA, ptB, pair, half)

                for j in range(nkv):
                    for _ in range(2):
                        nc.tensor.ldweights(weights=wq_sb[0][:, 0:128])
                    # row-tiled score pair: head A rows 0-63 -> tile (0,0),
                    # head B rows 64-127 -> tile (64,0); concurrent in PE
                    scA = scp.tile([128, SQH], dt.float32, name="scA", tag="scA")
                    scB = scp.tile([128, SQH], dt.float32, name="scB", tag="scB")
                    for t in range(2):
                        nc.tensor.matmul(
                            scA[:, ts(t, 512)],
                            kh_sb[pair][0:64, ts(j, 128)],
                            qh_sb[pair][0:64, half * SQH + t * 512:half * SQH + (t + 1) * 512],
                            start=True, stop=True,
                        )
                        nc.tensor.matmul(
                            scB[:, ts(t, 512)],
                            kh_sb[pair][64:128, ts(j, 128)],
                            qh_sb[pair][64:128, half * SQH + t * 512:half * SQH + (t + 1) * 512],
                            start=True, stop=True,
                        )
                    nc.scalar.activation(
                        ptA[j][:], scA[:], mybir.ActivationFunctionType.Exp,
                        bias=mb_sb[:, j:j + 1], scale=SCALE,
                    )
                    nc.scalar.activation(
                        ptB[j][:], scB[:], mybir.ActivationFunctionType.Exp,
                        bias=mb_sb[:, j:j + 1], scale=SCALE,
                    )
                    if prev_state is not None:
                        av_step(prev_state, 2 * j, a2h)
                        av_step(prev_state, 2 * j + 1, a2h)
                    # spread filler units across the slot's chunks
                    want = (j + 1) * len(fl) // nkv
                    while n_emitted < want:
                        fl[n_emitted]()
                        n_emitted += 1
                prev_state = cur_state
            else:
                # drain slot: AV for slot 7, then out-proj H1
                for k in range(2 * nkv):
                    av_step(prev_state, k, a2h)
                for f in fl:
                    f()

        for pool in (opool[0], npool[0], avp, scp, prj, ppool, pp):
            pool.release()

    nc.compile()
    _PROGRAM_CACHE[skv] = nc
    return nc


def make_in_maps(q, k, v, mask, Wq, Wk, Wv, Wo, skv):
    """Host-side shard/compact/transpose/cast. Returns per-core input dicts."""
    in_maps = []
    valid = mask != 0
    for core in range(N_CORES):
        b, hg = core // 2, core % 2
        idx = np.nonzero(valid[b])[0]
        cnt = len(idx)

        kc = np.zeros((skv, E), np.float32)
        vc = np.zeros((skv, E), np.float32)
        kc[:cnt] = k[b][idx]
        vc[:cnt] = v[b][idx]

        mbias = np.zeros((skv,), np.float32)
        mbias[cnt:] = NEG
        # [128, nkv]: column j = kv chunk j
        mb2 = np.ascontiguousarray(mbias.reshape(-1, 128).T)

        rows = slice(hg * DHC, (hg + 1) * DHC)
        in_maps.append(dict(
            qT=np.ascontiguousarray(q[b].T).astype(BF16),
            kT=np.ascontiguousarray(kc.T).astype(BF16),
            vT=np.ascontiguousarray(vc.T).astype(BF16),
            wqT=np.ascontiguousarray(Wq[rows, :].T).astype(BF16),
            wkT=np.ascontiguousarray(Wk[rows, :].T).astype(BF16),
            wvT=np.ascontiguousarray(Wv[rows, :].T).astype(BF16),
            woT=np.ascontiguousarray(Wo[:, rows].T).astype(BF16),
            mb=mb2,
        ))
    return in_maps


def _numpy_fallback(q, k, v, mask, Wq, bq, Wk, bk, Wv, bv, Wo, bo):
    out = np.zeros((B, SQ, E), np.float32)
    for b in range(B):
        qh = (q[b] @ Wq.T + bq).reshape(SQ, H_TOT, D).transpose(1, 0, 2)
        kh = (k[b] @ Wk.T + bk).reshape(-1, H_TOT, D).transpose(1, 0, 2)
        vh = (v[b] @ Wv.T + bv).reshape(-1, H_TOT, D).transpose(1, 0, 2)
        att = np.einsum("hqd,hkd->hqk", qh, kh) * SCALE
        valid = mask[b] != 0
        if not valid.any():
            out[b] = bo
            continue
        att = np.where(valid[None, None, :], att, -np.inf)
        att = att - att.max(-1, keepdims=True)
        att = np.exp(att)
        att /= att.sum(-1, keepdims=True)
        o = np.einsum("hqk,hkd->hqd", att, vh)
        o = o.transpose(1, 0, 2).reshape(SQ, E)
        out[b] = o @ Wo.T + bo
    return out


def kernel(**inputs):
    global LAST_RESULTS
    q = np.asarray(inputs["q"], np.float32)
    k = np.asarray(inputs["k"], np.float32)
    v = np.asarray(inputs["v"], np.float32)
    mask = np.asarray(inputs["mask"])
    Wq, bq = np.asarray(inputs["Wq"], np.float32), np.asarray(inputs["bq"], np.float32)
    Wk, bk = np.asarray(inputs["Wk"], np.float32), np.asarray(inputs["bk"], np.float32)
    Wv, bv = np.asarray(inputs["Wv"], np.float32), np.asarray(inputs["bv"], np.float32)
    Wo, bo = np.asarray(inputs["Wo"], np.float32), np.asarray(inputs["bo"], np.float32)

    if any(np.abs(x).max() > 0 for x in (bq, bk, bv)):
        # q/k/v biases are zero in this problem's setup; a nonzero bias
        # would need the augmented-contraction path, so fall back.
        return _numpy_fallback(q, k, v, mask, Wq, bq, Wk, bk, Wv, bv, Wo, bo)

    valid = mask != 0
    counts = valid.sum(axis=1)
    if counts.max() == 0:
        return np.broadcast_to(bo, (B, SQ, E)).astype(np.float32).copy()

    skv = int(-(-counts.max() // 128) * 128)
    nc = build_program(skv)
    in_maps = make_in_maps(q, k, v, mask, Wq, Wk, Wv, Wo, skv)

    res = bass_utils.run_bass_kernel_spmd(nc, in_maps, core_ids=list(range(N_CORES)))
    LAST_RESULTS = res

    out = np.empty((B, SQ, E), np.float32)
    for b in range(B):
        if counts[b] == 0:
            out[b] = bo
        else:
            p0 = res.results[2 * b]["outT"]
            p1 = res.results[2 * b + 1]["outT"]
            out[b] = p0.T + p1.T + bo
    return out


# revision 17
# speedup vs baseline: 1.5105x; 1.0106x over previous
"""Multi-head attention (batched, key-padding mask) Trainium2 Bass kernel — v2.

Problem: nn_MultiHeadBatched
  q,k,v: [B=4, S=2048, E=1024] fp32; mask: [B, 2048] int32 (key padding)
  16 heads, head_dim 64; torch-Linear style q/k/v/out projections.

Sharding (8 cores): core c handles batch b=c//2 and head group hg=c%2
(8 heads each).  q/k/v projections are column-parallel over the head
group; out-projection is row-parallel — each core produces a partial
[E, Sq] output and the host sums the two partials per batch (+ bo).

v2 structure (single NeuronCore program, SPMD over 8 cores):
  - Host compacts the KV sequence per batch to the valid (mask!=0)
    positions, pads to a multiple of 128 (SKV); padded positions get an
    additive -1e30 bias folded into the ScalarE exp activation.
  - All matmuls bf16 with fp32 PSUM accumulation.
  - Scores computed transposed ([kv, q]); softmax normalizer Z comes
    from an all-ones column appended to each head's V (row 64 of the AV
    accumulation).  No max-subtraction (scores/8 ~ N(0,1)).
  - Slot pipeline over (q-half, head-pair): slot s = (half s//4, pair
    s%4) computes scores+exp for its half/pair while the PE also runs
    AV for slot s-1, interleaved chunk by chunk.  Head pairs live in
    PE row groups 0-63 / 64-127, so the two heads' score matmuls are
    issued back-to-back with tile_position (0,0)/(64,0) and execute
    CONCURRENTLY in the PE array (row tiling) — 2x score throughput.
  - The attention phase is ScalarE(exp)-bound; all projections (Q, K,
    V, out) are emitted as filler units inside the slots' chunk loops
    so the PE does them in exp-wait slack.  Q input is loaded and
    projected per q-half to cut SBUF pressure.
  - 1/Z: the Z row [1, 1024] is spread across 128 partitions via a
    SBUF->SBUF DMA, reciprocal'd as [128, 8] (DVE reciprocal is ~8
    cycles/elem — free-dim size is what costs), and DMA'd back.
"""

import os
import sys

import numpy as np

sys.path.insert(0, "/opt/trn_rl_repo")

import concourse.bass as bass
import concourse.bacc as bacc
import concourse.mybir as mybir
import concourse.tile as tile
from concourse import bass_utils

import ml_dtypes

BF16 = ml_dtypes.bfloat16

B, SQ, E = 4, 2048, 1024
H_TOT, D = 16, 64
HPC = H_TOT // 2            # heads per core (head-group split in 2)
DHC = HPC * D               # 512 projected channels per core
NE = E // 128               # contraction chunks
NDH = DHC // 128            # dh chunks per core
SQH = SQ // 2               # q-half width
NEG = -1.0e30
SCALE = D ** -0.5

N_CORES = 8

_PROGRAM_CACHE = {}
LAST_RESULTS = None


def _chunks512(n):
    out = []
    o = 0
    while o < n:
        w = min(512, n - o)
        out.append((o, w))
        o += w
    return out


def build_program(skv):
    """Build + compile the single-core SPMD Bass program for padded KV
    length `skv` (multiple of 128)."""
    if skv in _PROGRAM_CACHE:
        return _PROGRAM_CACHE[skv]

    nkv = skv // 128
    dt = mybir.dt

    nc = bacc.Bacc(
        "TRN2",
        target_bir_lowering=False,
        debug=False,
        enable_asserts=False,
        num_devices=N_CORES,
    )

    # DRAM I/O (per-core shapes)
    qT = nc.dram_tensor("qT", [E, SQ], dt.bfloat16, kind="ExternalInput").ap()
    kT = nc.dram_tensor("kT", [E, skv], dt.bfloat16, kind="ExternalInput").ap()
    vT = nc.dram_tensor("vT", [E, skv], dt.bfloat16, kind="ExternalInput").ap()
    wqT = nc.dram_tensor("wqT", [E, DHC], dt.bfloat16, kind="ExternalInput").ap()
    wkT = nc.dram_tensor("wkT", [E, DHC], dt.bfloat16, kind="ExternalInput").ap()
    wvT = nc.dram_tensor("wvT", [E, DHC], dt.bfloat16, kind="ExternalInput").ap()
    woT = nc.dram_tensor("woT", [DHC, E], dt.bfloat16, kind="ExternalInput").ap()
    mb = nc.dram_tensor("mb", [128, nkv], dt.float32, kind="ExternalInput").ap()
    outT = nc.dram_tensor("outT", [E, SQ], dt.float32, kind="ExternalOutput").ap()

    ts = bass.ts
    kvchunks = _chunks512(skv)

    with tile.TileContext(nc) as tc:
        pp = tc.alloc_tile_pool(name="persist", bufs=1)

        # Persistent SBUF tensors
        wq_sb = [pp.tile([128, DHC], dt.bfloat16, name=f"wq{e}", tag=f"wq{e}") for e in range(NE)]
        wk_sb = [pp.tile([128, DHC], dt.bfloat16, name=f"wk{e}", tag=f"wk{e}") for e in range(NE)]
        wv_sb = [pp.tile([128, DHC], dt.bfloat16, name=f"wv{e}", tag=f"wv{e}") for e in range(NE)]
        wo_sb = [pp.tile([128, E], dt.bfloat16, name=f"wo{c}", tag=f"wo{c}") for c in range(NDH)]
        # qh/aall split per q-half: separate tiles kill false WAR deps
        # between one half's reads and the other half's writes.
        qh_sb = [[pp.tile([128, SQH], dt.bfloat16, name=f"qh{h}_{c}", tag=f"qh{h}_{c}") for c in range(NDH)]
                 for h in range(2)]
        kh_sb = [pp.tile([128, skv], dt.bfloat16, name=f"kh{c}", tag=f"kh{c}") for c in range(NDH)]
        # V with per-head interleaved ones column: [kv, 8*(64+1)]
        va_sb = [pp.tile([128, HPC * (D + 1)], dt.bfloat16, name=f"va{j}", tag=f"va{j}") for j in range(nkv)]
        aall_sb = [[pp.tile([128, SQH], dt.bfloat16, name=f"aall{h}_{c}", tag=f"aall{h}_{c}") for c in range(NDH)]
                   for h in range(2)]
        mb_sb = pp.tile([128, nkv], dt.float32, name="mbt", tag="mbt")

        for j in range(nkv):
            nc.gpsimd.memset(va_sb[j][:, D::D + 1], 1.0)

        # Input pools (released as the projections complete; right-side
        # stack so mid-stream release doesn't violate LIFO pool order)
        qip = tc.alloc_tile_pool(name="qinp", bufs=1, side="right")
        kip = tc.alloc_tile_pool(name="kinp", bufs=1, side="right")
        vip = tc.alloc_tile_pool(name="vinp", bufs=1, side="right")

        # P tiles: per slot, 2 heads x nkv chunks of [128, SQH] bf16
        ppool = tc.alloc_tile_pool(name="ppool", bufs=2)

        # PSUM pools: prj 2 banks + scores 4 banks + AV 2 banks = 8
        prj = tc.alloc_tile_pool(name="prj", bufs=2, space="PSUM")
        scp = tc.alloc_tile_pool(name="scp", bufs=1, space="PSUM")
        avp = tc.alloc_tile_pool(name="avp", bufs=1, space="PSUM")

        npool = [None]  # allocated after vip release
        opool = [None]  # allocated after qip/kip release

        # ---------------- input DMAs ----------------
        q1_sb = [qip.tile([128, SQH], dt.bfloat16, name=f"q{e}", tag=f"q{e}") for e in range(NE)]
        k_sb = [kip.tile([128, skv], dt.bfloat16, name=f"k{e}", tag=f"k{e}") for e in range(NE)]
        v_sb = [vip.tile([128, skv], dt.bfloat16, name=f"v{e}", tag=f"v{e}") for e in range(NE)]
        # DMA order matches first-use order: K-proj(0) first, then exp's
        # mask bias, Q-proj(0, H0), then slot-0's V-proj fillers.
        for e in range(NE):
            nc.sync.dma_start(wk_sb[e][:], wkT[ts(e, 128), :])
            nc.sync.dma_start(k_sb[e][:], kT[ts(e, 128), :])
        nc.sync.dma_start(mb_sb[:], mb[:])
        for e in range(NE):
            nc.sync.dma_start(wq_sb[e][:], wqT[ts(e, 128), :])
            nc.sync.dma_start(q1_sb[e][:], qT[ts(e, 128), 0:SQH])
        for e in range(NE):
            nc.sync.dma_start(wv_sb[e][:], wvT[ts(e, 128), :])
            nc.sync.dma_start(v_sb[e][:], vT[ts(e, 128), :])
        for c in range(NDH):
            nc.sync.dma_start(wo_sb[c][:], woT[ts(c, 128), :])

        # ---------------- projection / out-proj unit emitters ----------------
        def kproj_unit(c, o, w):
            def emit():
                kps = prj.tile([128, 512], dt.float32, name="kps", tag="prj")
                for e in range(NE):
                    nc.tensor.matmul(
                        kps[:, 0:w], wk_sb[e][:, ts(c, 128)], k_sb[e][:, o:o + w],
                        start=(e == 0), stop=(e == NE - 1),
                    )
                nc.vector.tensor_copy(kh_sb[c][:, o:o + w], kps[:, 0:w])
            return emit

        def qproj_unit(c, half, t, q_tiles):
            def emit():
                qps = prj.tile([128, 512], dt.float32, name="qps", tag="prj")
                for e in range(NE):
                    nc.tensor.matmul(
                        qps[:], wq_sb[e][:, ts(c, 128)], q_tiles[e][:, ts(t, 512)],
                        start=(e == 0), stop=(e == NE - 1),
                    )
                nc.vector.tensor_copy(qh_sb[half][c][:, ts(t, 512)], qps[:])
            return emit

        def vproj_unit(j):
            def emit():
                vps = prj.tile([128, DHC], dt.float32, name="vps", tag="prj")
                for e in range(NE):
                    nc.tensor.matmul(
                        vps[:], v_sb[e][:, ts(j, 128)], wv_sb[e][:],
                        start=(e == 0), stop=(e == NE - 1),
                    )
                dst = va_sb[j].rearrange("p (h x) -> p h x", x=D + 1)[:, :, 0:D]
                src = vps.rearrange("p (h x) -> p h x", x=D)
                nc.vector.tensor_copy(dst, src)
            return emit

        q2_holder = {}

        def qdma2_unit():
            def emit():
                q2 = [qip.tile([128, SQH], dt.bfloat16, name=f"q{e}", tag=f"q{e}") for e in range(NE)]
                for e in range(NE):
                    nc.sync.dma_start(q2[e][:], qT[ts(e, 128), SQH:SQ])
                q2_holder["t"] = q2
            return emit

        def qproj2_unit(c, t):
            def emit():
                qps = prj.tile([128, 512], dt.float32, name="qps", tag="prj")
                for e in range(NE):
                    nc.tensor.matmul(
                        qps[:], wq_sb[e][:, ts(c, 128)], q2_holder["t"][e][:, ts(t, 512)],
                        start=(e == 0), stop=(e == NE - 1),
                    )
                nc.vector.tensor_copy(qh_sb[1][c][:, ts(t, 512)], qps[:])
            return emit

        def oproj_unit(half, eo, t):
            def emit():
                ops = prj.tile([128, 512], dt.float32, name="ops", tag="prj")
                for c in range(NDH):
                    nc.tensor.matmul(
                        ops[:], wo_sb[c][:, ts(eo, 128)],
                        aall_sb[half][c][:, ts(t, 512)],
                        start=(c == 0), stop=(c == NDH - 1),
                    )
                ob = opool[0].tile([128, 512], dt.float32, name="ob", tag="ob", bufs=4)
                nc.vector.tensor_copy(ob[:], ops[:])
                nc.sync.dma_start(
                    outT[ts(eo, 128), half * SQH + t * 512:half * SQH + (t + 1) * 512], ob[:])
            return emit

        # ---------------- normalization ----------------
        ZW = SQH // 128  # 8

        def emit_norm(hp, half, a2):
            c, r = hp // 2, hp % 2
            np_ = npool[0]
            au = np_.tile([D + 1, SQH], dt.float32, name="au", tag="au", bufs=2)
            nc.vector.tensor_copy(au[:], a2[:])
            # spread Z across partitions; reciprocal cost is free-dim-size bound
            zt = np_.tile([128, ZW], dt.float32, name="zt", tag="zt", bufs=2)
            nc.sync.dma_start(zt[:], au[D:D + 1, :])
            rz8 = np_.tile([128, ZW], dt.float32, name="rz8", tag="rz8", bufs=2)
            nc.vector.reciprocal(rz8[:], zt[:])
            rzr = np_.tile([1, SQH], dt.float32, name="rzr", tag="rzr", bufs=2)
            nc.sync.dma_start(rzr[:], rz8[:])
            rb = np_.tile([D, SQH], dt.float32, name="rb", tag="rb", bufs=2)
            nc.gpsimd.partition_broadcast(rb[:], rzr[:])
            nc.gpsimd.tensor_mul(
                aall_sb[half][c][r * 64:(r + 1) * 64, :],
                au[0:D, :], rb[:],
            )

        # ---------------- AV machinery ----------------
        def av_step(state, k, a2h):
            ptA, ptB, pair, half = state
            local = 0 if k < nkv else 1
            j = k - nkv * local
            pts = ptA if local == 0 else ptB
            hp = 2 * pair + local
            if j == 0:
                a2h[0] = avp.tile([D + 1, SQH], dt.float32, name="a2", tag="a2")
            a2 = a2h[0]
            for t in range(2):
                nc.tensor.matmul(
                    a2[:, ts(t, 512)],
                    va_sb[j][:, hp * (D + 1):(hp + 1) * (D + 1)],
                    pts[j][:, ts(t, 512)],
                    start=(j == 0), stop=(j == nkv - 1),
                )
            if j == nkv - 1:
                emit_norm(hp, half, a2)

        # ---------------- filler schedule ----------------
        fillers = {s: [] for s in range(9)}
        # slot 0: K(1)/Q(1,H0) early (needed at slot 1 scores), V units
        # interleaved (V chunk j needed by slot 1's AV around chunk j/2).
        s0 = [vproj_unit(j) for j in range(nkv)]
        pri = [kproj_unit(1, o, w) for (o, w) in kvchunks]
        pri += [qproj_unit(1, 0, t, q1_sb) for t in range(2)]
        for i, u in enumerate(pri):
            s0.insert(2 * i + 1, u)
        fillers[0] = s0
        for (o, w) in kvchunks:
            fillers[1].append(kproj_unit(2, o, w))
        fillers[1] += [qproj_unit(2, 0, t, q1_sb) for t in range(2)]
        for (o, w) in kvchunks:
            fillers[2].append(kproj_unit(3, o, w))
        fillers[2] += [qproj_unit(3, 0, t, q1_sb) for t in range(2)]
        fillers[2] += [qdma2_unit()]
        fillers[2] += [qproj2_unit(0, t) for t in range(2)]
        fillers[2] += [qproj2_unit(1, t) for t in range(2)]
        fillers[3] += [qproj2_unit(2, t) for t in range(2)]
        fillers[3] += [qproj2_unit(3, t) for t in range(2)]
        oh0 = [oproj_unit(0, eo, t) for eo in range(NE) for t in range(2)]
        fillers[5] = oh0[0:6]
        fillers[6] = oh0[6:11]
        fillers[7] = oh0[11:16]
        fillers[8] = [oproj_unit(1, eo, t) for eo in range(NE) for t in range(2)]

        # ---------------- lead-in: K(0), Q(0, H0) ----------------
        for (o, w) in kvchunks:
            kproj_unit(0, o, w)()
        for t in range(2):
            qproj_unit(0, 0, t, q1_sb)()

        # ---------------- slot loop ----------------
        prev_state = None
        for s in range(9):
            if s == 1:
                vip.release()
                npool[0] = tc.alloc_tile_pool(name="norm", bufs=1)
            if s == 4:
                kip.release()
                qip.release()
                opool[0] = tc.alloc_tile_pool(name="outp", bufs=1)

            fl = list(fillers[s])
            n_emitted = 0
            a2h = [None]

            if s < 8:
                half, pair = s // 4, s % 4
                # pA single-buffered: the consuming AV step 2j of the next
                # slot runs at chunk j//2 <= j, before exp(j) needs the buf.
                # pB double-buffered: its AV steps run in the slot's 2nd half.
                ptA = [ppool.tile([128, SQH], dt.bfloat16, name=f"pA{j}", tag=f"pA{j}", bufs=1) for j in range(nkv)]
                ptB = [ppool.tile([128, SQH], dt.bfloat16, name=f"pB{j}", tag=f"pB{j}", bufs=2) for j in range(nkv)]
                cur_state = (ptA, ptB, pair, half)

                for j in range(nkv):
                    nc.tensor.ldweights(weights=wq_sb[0][:, 0:128])
                    # row-tiled score pair: head A rows 0-63 -> tile (0,0),
                    # head B rows 64-127 -> tile (64,0); concurrent in PE
                    scA = scp.tile([128, SQH], dt.float32, name="scA", tag="scA")
                    scB = scp.tile([128, SQH], dt.float32, name="scB", tag="scB")
                    for t in range(2):
                        nc.tensor.matmul(
                            scA[:, ts(t, 512)],
                            kh_sb[pair][0:64, ts(j, 128)],
                            qh_sb[half][pair][0:64, ts(t, 512)],
                            start=True, stop=True,
                        )
                        nc.tensor.matmul(
                            scB[:, ts(t, 512)],
                            kh_sb[pair][64:128, ts(j, 128)],
                            qh_sb[half][pair][64:128, ts(t, 512)],
                            start=True, stop=True,
                        )
                    nc.scalar.activation(
                        ptA[j][:], scA[:], mybir.ActivationFunctionType.Exp,
                        bias=mb_sb[:, j:j + 1], scale=SCALE,
                    )
                    nc.scalar.activation(
                        ptB[j][:], scB[:], mybir.ActivationFunctionType.Exp,
                        bias=mb_sb[:, j:j + 1], scale=SCALE,
                    )
                    if prev_state is not None:
                        av_step(prev_state, 2 * j, a2h)
                        av_step(prev_state, 2 * j + 1, a2h)
                    # spread filler units across the slot's chunks
                    want = (j + 1) * len(fl) // nkv
                    while n_emitted < want:
                        fl[n_emitted]()
                        n_emitted += 1
                prev_state = cur_state
            else:
                # drain slot: AV for slot 7, then out-proj H1
                for k in range(2 * nkv):
                    av_step(prev_state, k, a2h)
                for f in fl:
                    f()

        for pool in (opool[0], npool[0], avp, scp, prj, ppool, pp):
            pool.release()

    nc.compile()
    _PROGRAM_CACHE[skv] = nc
    return nc


def make_in_maps(q, k, v, mask, Wq, Wk, Wv, Wo, skv):
    """Host-side shard/compact/transpose/cast. Returns per-core input dicts."""
    in_maps = []
    valid = mask != 0
    for core in range(N_CORES):
        b, hg = core // 2, core % 2
        idx = np.nonzero(valid[b])[0]
        cnt = len(idx)

        kc = np.zeros((skv, E), np.float32)
        vc = np.zeros((skv, E), np.float32)
        kc[:cnt] = k[b][idx]
        vc[:cnt] = v[b][idx]

        mbias = np.zeros((skv,), np.float32)
        mbias[cnt:] = NEG
        # [128, nkv]: column j = kv chunk j
        mb2 = np.ascontiguousarray(mbias.reshape(-1, 128).T)

        rows = slice(hg * DHC, (hg + 1) * DHC)
        in_maps.append(dict(
            qT=np.ascontiguousarray(q[b].T).astype(BF16),
            kT=np.ascontiguousarray(kc.T).astype(BF16),
            vT=np.ascontiguousarray(vc.T).astype(BF16),
            wqT=np.ascontiguousarray(Wq[rows, :].T).astype(BF16),
            wkT=np.ascontiguousarray(Wk[rows, :].T).astype(BF16),
            wvT=np.ascontiguousarray(Wv[rows, :].T).astype(BF16),
            woT=np.ascontiguousarray(Wo[:, rows].T).astype(BF16),
            mb=mb2,
        ))
    return in_maps


def _numpy_fallback(q, k, v, mask, Wq, bq, Wk, bk, Wv, bv, Wo, bo):
    out = np.zeros((B, SQ, E), np.float32)
    for b in range(B):
        qh = (q[b] @ Wq.T + bq).reshape(SQ, H_TOT, D).transpose(1, 0, 2)
        kh = (k[b] @ Wk.T + bk).reshape(-1, H_TOT, D).transpose(1, 0, 2)
        vh = (v[b] @ Wv.T + bv).reshape(-1, H_TOT, D).transpose(1, 0, 2)
        att = np.einsum("hqd,hkd->hqk", qh, kh) * SCALE
        valid = mask[b] != 0
        if not valid.any():
            out[b] = bo
            continue
        att = np.where(valid[None, None, :], att, -np.inf)
        att = att - att.max(-1, keepdims=True)
        att = np.exp(att)
        att /= att.sum(-1, keepdims=True)
        o = np.einsum("hqk,hkd->hqd", att, vh)
        o = o.transpose(1, 0, 2).reshape(SQ, E)
        out[b] = o @ Wo.T + bo
    return out


def kernel(**inputs):
    global LAST_RESULTS
    q = np.asarray(inputs["q"], np.float32)
    k = np.asarray(inputs["k"], np.float32)
    v = np.asarray(inputs["v"], np.float32)
    mask = np.asarray(inputs["mask"])
    Wq, bq = np.asarray(inputs["Wq"], np.float32), np.asarray(inputs["bq"], np.float32)
    Wk, bk = np.asarray(inputs["Wk"], np.float32), np.asarray(inputs["bk"], np.float32)
    Wv, bv = np.asarray(inputs["Wv"], np.float32), np.asarray(inputs["bv"], np.float32)
    Wo, bo = np.asarray(inputs["Wo"], np.float32), np.asarray(inputs["bo"], np.float32)

    if any(np.abs(x).max() > 0 for x in (bq, bk, bv)):
        # q/k/v biases are zero in this problem's setup; a nonzero bias
        # would need the augmented-contraction path, so fall back.
        return _numpy_fallback(q, k, v, mask, Wq, bq, Wk, bk, Wv, bv, Wo, bo)

    valid = mask != 0
    counts = valid.sum(axis=1)
    if counts.max() == 0:
        return np.broadcast_to(bo, (B, SQ, E)).astype(np.float32).copy()

    skv = int(-(-counts.max() // 128) * 128)
    nc = build_program(skv)
    in_maps = make_in_maps(q, k, v, mask, Wq, Wk, Wv, Wo, skv)

    res = bass_utils.run_bass_kernel_spmd(nc, in_maps, core_ids=list(range(N_CORES)))
    LAST_RESULTS = res

    out = np.empty((B, SQ, E), np.float32)
    for b in range(B):
        if counts[b] == 0:
            out[b] = bo
        else:
            p0 = res.results[2 * b]["outT"]
            p1 = res.results[2 * b + 1]["outT"]
            out[b] = p0.T + p1.T + bo
    return out


# revision 22
# speedup vs baseline: 2.0453x; 1.3541x over previous
"""Multi-head attention (batched, key-padding mask) Trainium2 Bass kernel — v2.

Problem: nn_MultiHeadBatched
  q,k,v: [B=4, S=2048, E=1024] fp32; mask: [B, 2048] int32 (key padding)
  16 heads, head_dim 64; torch-Linear style q/k/v/out projections.

Sharding (8 cores): core c handles batch b=c//2 and head group hg=c%2
(8 heads each).  q/k/v projections are column-parallel over the head
group; out-projection is row-parallel — each core produces a partial
[E, Sq] output and the host sums the two partials per batch (+ bo).

v2 structure (single NeuronCore program, SPMD over 8 cores):
  - Host compacts the KV sequence per batch to the valid (mask!=0)
    positions, pads to a multiple of 128 (SKV); padded positions get an
    additive -1e30 bias folded into the ScalarE exp activation.
  - All matmuls bf16 with fp32 PSUM accumulation.
  - Scores computed transposed ([kv, q]); softmax normalizer Z comes
    from an all-ones column appended to each head's V (row 64 of the AV
    accumulation).  No max-subtraction (scores/8 ~ N(0,1)).
  - Slot pipeline over (q-half, head-pair): slot s = (half s//4, pair
    s%4) computes scores+exp for its half/pair while the PE also runs
    AV for slot s-1, interleaved chunk by chunk.  Head pairs live in
    PE row groups 0-63 / 64-127, so the two heads' score matmuls are
    issued back-to-back with tile_position (0,0)/(64,0) and execute
    CONCURRENTLY in the PE array (row tiling) — 2x score throughput.
  - The attention phase is ScalarE(exp)-bound; all projections (Q, K,
    V, out) are emitted as filler units inside the slots' chunk loops
    so the PE does them in exp-wait slack.  Q input is loaded and
    projected per q-half to cut SBUF pressure.
  - 1/Z: the Z row [1, 1024] is spread across 128 partitions via a
    SBUF->SBUF DMA, reciprocal'd as [128, 8] (DVE reciprocal is ~8
    cycles/elem — free-dim size is what costs), and DMA'd back.
"""

import os
import sys

import numpy as np

sys.path.insert(0, "/opt/trn_rl_repo")

import concourse.bass as bass
import concourse.bacc as bacc
import concourse.mybir as mybir
import concourse.tile as tile
from concourse import bass_utils

import ml_dtypes

BF16 = ml_dtypes.bfloat16

B, SQ, E = 4, 2048, 1024
H_TOT, D = 16, 64
HPC = H_TOT // 2            # heads per core (head-group split in 2)
DHC = HPC * D               # 512 projected channels per core
NE = E // 128               # contraction chunks
NDH = DHC // 128            # dh chunks per core
SQH = SQ // 2               # q-half width
NEG = -1.0e30
SCALE = D ** -0.5

N_CORES = 8

_PROGRAM_CACHE = {}
LAST_RESULTS = None


def _chunks512(n):
    out = []
    o = 0
    while o < n:
        w = min(512, n - o)
        out.append((o, w))
        o += w
    return out


def build_program(skv):
    """Build + compile the single-core SPMD Bass program for padded KV
    length `skv` (multiple of 128)."""
    if skv in _PROGRAM_CACHE:
        return _PROGRAM_CACHE[skv]

    nkv = skv // 128
    dt = mybir.dt

    nc = bacc.Bacc(
        "TRN2",
        target_bir_lowering=False,
        debug=False,
        enable_asserts=False,
        num_devices=N_CORES,
    )

    # DRAM I/O (per-core shapes)
    qT = nc.dram_tensor("qT", [E, SQ], dt.bfloat16, kind="ExternalInput").ap()
    kT = nc.dram_tensor("kT", [E, skv], dt.bfloat16, kind="ExternalInput").ap()
    vT = nc.dram_tensor("vT", [E, skv], dt.bfloat16, kind="ExternalInput").ap()
    wqT = nc.dram_tensor("wqT", [E, DHC], dt.bfloat16, kind="ExternalInput").ap()
    wkT = nc.dram_tensor("wkT", [E, DHC], dt.bfloat16, kind="ExternalInput").ap()
    wvT = nc.dram_tensor("wvT", [E, DHC], dt.bfloat16, kind="ExternalInput").ap()
    woT = nc.dram_tensor("woT", [DHC, E], dt.bfloat16, kind="ExternalInput").ap()
    mb = nc.dram_tensor("mb", [128, nkv], dt.float32, kind="ExternalInput").ap()
    outT = nc.dram_tensor("outT", [E, SQ], dt.float32, kind="ExternalOutput").ap()

    ts = bass.ts
    kvchunks = _chunks512(skv)

    with tile.TileContext(nc) as tc:
        pp = tc.alloc_tile_pool(name="persist", bufs=1)

        # Persistent SBUF tensors
        wq_sb = [pp.tile([128, DHC], dt.bfloat16, name=f"wq{e}", tag=f"wq{e}") for e in range(NE)]
        wk_sb = [pp.tile([128, DHC], dt.bfloat16, name=f"wk{e}", tag=f"wk{e}") for e in range(NE)]
        wv_sb = [pp.tile([128, DHC], dt.bfloat16, name=f"wv{e}", tag=f"wv{e}") for e in range(NE)]
        wo_sb = [pp.tile([128, E], dt.bfloat16, name=f"wo{c}", tag=f"wo{c}") for c in range(NDH)]
        # qh/aall split per q-half: separate tiles kill false WAR deps
        # between one half's reads and the other half's writes.
        qh_sb = [[pp.tile([128, SQH], dt.bfloat16, name=f"qh{h}_{c}", tag=f"qh{h}_{c}") for c in range(NDH)]
                 for h in range(2)]
        kh_sb = [pp.tile([128, skv], dt.bfloat16, name=f"kh{c}", tag=f"kh{c}") for c in range(NDH)]
        # V with per-head interleaved ones column: [kv, 8*(64+1)]
        va_sb = [pp.tile([128, HPC * (D + 1)], dt.bfloat16, name=f"va{j}", tag=f"va{j}") for j in range(nkv)]
        aall_sb = [[pp.tile([128, SQH], dt.bfloat16, name=f"aall{h}_{c}", tag=f"aall{h}_{c}") for c in range(NDH)]
                   for h in range(2)]
        mb_sb = pp.tile([128, nkv], dt.float32, name="mbt", tag="mbt")

        for j in range(nkv):
            nc.gpsimd.memset(va_sb[j][:, D::D + 1], 1.0)

        # Input pools (released as the projections complete; right-side
        # stack so mid-stream release doesn't violate LIFO pool order)
        qip = tc.alloc_tile_pool(name="qinp", bufs=1, side="right")
        kip = tc.alloc_tile_pool(name="kinp", bufs=1, side="right")
        vip = tc.alloc_tile_pool(name="vinp", bufs=1, side="right")

        # P tiles: per slot, 2 heads x nkv chunks of [128, SQH] bf16
        ppool = tc.alloc_tile_pool(name="ppool", bufs=2)

        # PSUM pools: prj 2 banks + scores 4 banks + AV 2 banks = 8
        prj = tc.alloc_tile_pool(name="prj", bufs=2, space="PSUM")
        scp = tc.alloc_tile_pool(name="scp", bufs=1, space="PSUM")
        avp = tc.alloc_tile_pool(name="avp", bufs=1, space="PSUM")

        npool = [None]  # allocated after vip release
        opool = [None]  # allocated after qip/kip release

        # ---------------- input DMAs ----------------
        q1_sb = [qip.tile([128, SQH], dt.bfloat16, name=f"q{e}", tag=f"q{e}") for e in range(NE)]
        k_sb = [kip.tile([128, skv], dt.bfloat16, name=f"k{e}", tag=f"k{e}") for e in range(NE)]
        v_sb = [vip.tile([128, skv], dt.bfloat16, name=f"v{e}", tag=f"v{e}") for e in range(NE)]
        # DMA order matches first-use order: K-proj(0) first, then exp's
        # mask bias, Q-proj(0, H0), then slot-0's V-proj fillers.
        for e in range(NE):
            nc.sync.dma_start(wk_sb[e][:], wkT[ts(e, 128), :])
            nc.sync.dma_start(k_sb[e][:], kT[ts(e, 128), :])
        nc.sync.dma_start(mb_sb[:], mb[:])
        for e in range(NE):
            nc.sync.dma_start(wq_sb[e][:], wqT[ts(e, 128), :])
            nc.sync.dma_start(q1_sb[e][:], qT[ts(e, 128), 0:SQH])
        for e in range(NE):
            nc.sync.dma_start(wv_sb[e][:], wvT[ts(e, 128), :])
            nc.sync.dma_start(v_sb[e][:], vT[ts(e, 128), :])
        for c in range(NDH):
            nc.sync.dma_start(wo_sb[c][:], woT[ts(c, 128), :])

        # ---------------- projection / out-proj unit emitters ----------------
        def kproj_unit(c, o, w):
            def emit():
                kps = prj.tile([128, 512], dt.float32, name="kps", tag="prj")
                for e in range(NE):
                    nc.tensor.matmul(
                        kps[:, 0:w], wk_sb[e][:, ts(c, 128)], k_sb[e][:, o:o + w],
                        start=(e == 0), stop=(e == NE - 1),
                    )
                nc.vector.tensor_copy(kh_sb[c][:, o:o + w], kps[:, 0:w])
            return emit

        def qproj_unit(c, half, t, q_tiles):
            def emit():
                qps = prj.tile([128, 512], dt.float32, name="qps", tag="prj")
                for e in range(NE):
                    nc.tensor.matmul(
                        qps[:], wq_sb[e][:, ts(c, 128)], q_tiles[e][:, ts(t, 512)],
                        start=(e == 0), stop=(e == NE - 1),
                    )
                nc.vector.tensor_copy(qh_sb[half][c][:, ts(t, 512)], qps[:])
            return emit

        def vproj_unit(j):
            def emit():
                vps = prj.tile([128, DHC], dt.float32, name="vps", tag="prj")
                for e in range(NE):
                    nc.tensor.matmul(
                        vps[:], v_sb[e][:, ts(j, 128)], wv_sb[e][:],
                        start=(e == 0), stop=(e == NE - 1),
                    )
                dst = va_sb[j].rearrange("p (h x) -> p h x", x=D + 1)[:, :, 0:D]
                src = vps.rearrange("p (h x) -> p h x", x=D)
                nc.vector.tensor_copy(dst, src)
            return emit

        q2_holder = {}

        def qdma2_unit():
            def emit():
                q2 = [qip.tile([128, SQH], dt.bfloat16, name=f"q{e}", tag=f"q{e}") for e in range(NE)]
                for e in range(NE):
                    nc.sync.dma_start(q2[e][:], qT[ts(e, 128), SQH:SQ])
                q2_holder["t"] = q2
            return emit

        def qproj2_unit(c, t):
            def emit():
                qps = prj.tile([128, 512], dt.float32, name="qps", tag="prj")
                for e in range(NE):
                    nc.tensor.matmul(
                        qps[:], wq_sb[e][:, ts(c, 128)], q2_holder["t"][e][:, ts(t, 512)],
                        start=(e == 0), stop=(e == NE - 1),
                    )
                nc.vector.tensor_copy(qh_sb[1][c][:, ts(t, 512)], qps[:])
            return emit

        def oproj_unit(half, eo, t):
            def emit():
                ops = prj.tile([128, 512], dt.float32, name="ops", tag="prj")
                for c in range(NDH):
                    nc.tensor.matmul(
                        ops[:], wo_sb[c][:, ts(eo, 128)],
                        aall_sb[half][c][:, ts(t, 512)],
                        start=(c == 0), stop=(c == NDH - 1),
                    )
                ob = opool[0].tile([128, 512], dt.float32, name="ob", tag="ob", bufs=4)
                nc.vector.tensor_copy(ob[:], ops[:])
                nc.sync.dma_start(
                    outT[ts(eo, 128), half * SQH + t * 512:half * SQH + (t + 1) * 512], ob[:])
            return emit

        # ---------------- normalization ----------------
        ZW = SQH // 128  # 8

        def emit_norm(hp, half, a2):
            c, r = hp // 2, hp % 2
            np_ = npool[0]
            au = np_.tile([D + 1, SQH], dt.float32, name="au", tag="au", bufs=3)
            nc.vector.tensor_copy(au[:], a2[:])
            # spread Z across partitions; reciprocal cost is free-dim-size bound
            zt = np_.tile([128, ZW], dt.float32, name="zt", tag="zt", bufs=2)
            nc.sync.dma_start(zt[:], au[D:D + 1, :])
            rz8 = np_.tile([128, ZW], dt.float32, name="rz8", tag="rz8", bufs=2)
            nc.vector.reciprocal(rz8[:], zt[:])
            rzr = np_.tile([1, SQH], dt.float32, name="rzr", tag="rzr", bufs=2)
            nc.sync.dma_start(rzr[:], rz8[:])
            rb = np_.tile([D, SQH], dt.float32, name="rb", tag="rb", bufs=2)
            nc.gpsimd.partition_broadcast(rb[:], rzr[:])
            # mul on DVE, not gpsimd: keeping Pool broadcast-only avoids the
            # ~8us Q7 library reload between broadcast and multiply kernels.
            nc.vector.tensor_mul(
                aall_sb[half][c][r * 64:(r + 1) * 64, :],
                au[0:D, :], rb[:],
            )

        # ---------------- AV machinery ----------------
        def av_step(state, k, a2h):
            ptA, ptB, pair, half = state
            local = 0 if k < nkv else 1
            j = k - nkv * local
            pts = ptA if local == 0 else ptB
            hp = 2 * pair + local
            if j == 0:
                a2h[0] = avp.tile([D + 1, SQH], dt.float32, name="a2", tag="a2")
            a2 = a2h[0]
            for t in range(2):
                nc.tensor.matmul(
                    a2[:, ts(t, 512)],
                    va_sb[j][:, hp * (D + 1):(hp + 1) * (D + 1)],
                    pts[j][:, ts(t, 512)],
                    start=(j == 0), stop=(j == nkv - 1),
                )
            if j == nkv - 1:
                emit_norm(hp, half, a2)

        # ---------------- filler schedule ----------------
        fillers = {s: [] for s in range(9)}
        # slot 0: K(1)/Q(1,H0) early (needed at slot 1 scores), V units
        # interleaved (V chunk j needed by slot 1's AV around chunk j/2).
        s0 = [vproj_unit(j) for j in range(nkv)]
        pri = [kproj_unit(1, o, w) for (o, w) in kvchunks]
        pri += [qproj_unit(1, 0, t, q1_sb) for t in range(2)]
        for i, u in enumerate(pri):
            s0.insert(2 * i + 1, u)
        fillers[0] = s0
        for (o, w) in kvchunks:
            fillers[1].append(kproj_unit(2, o, w))
        fillers[1] += [qproj_unit(2, 0, t, q1_sb) for t in range(2)]
        for (o, w) in kvchunks:
            fillers[2].append(kproj_unit(3, o, w))
        fillers[2] += [qproj_unit(3, 0, t, q1_sb) for t in range(2)]
        fillers[2] += [qdma2_unit()]
        fillers[2] += [qproj2_unit(0, t) for t in range(2)]
        fillers[2] += [qproj2_unit(1, t) for t in range(2)]
        fillers[3] += [qproj2_unit(2, t) for t in range(2)]
        fillers[3] += [qproj2_unit(3, t) for t in range(2)]
        oh0 = [oproj_unit(0, eo, t) for eo in range(NE) for t in range(2)]
        fillers[5] = oh0[0:5]
        fillers[6] = oh0[5:10]
        fillers[7] = oh0[10:16]
        fillers[8] = [oproj_unit(1, eo, t) for eo in range(NE) for t in range(2)]

        def warm_mm():
            # dependency-free matmul into a dead PSUM tile: keeps the PE
            # HAM-busy through norm-chain waits so the tail runs at 2.4GHz
            wps = scp.tile([128, SQH], dt.float32, name="scA", tag="scA")
            nc.tensor.matmul(
                wps[:, 0:512], kh_sb[0][0:64, 0:128], qh_sb[1][0][0:64, 0:512],
                start=True, stop=True,
            )

        # ---------------- lead-in: K(0), Q(0, H0) ----------------
        for (o, w) in kvchunks:
            kproj_unit(0, o, w)()
        for t in range(2):
            qproj_unit(0, 0, t, q1_sb)()

        # ---------------- slot loop ----------------
        prev_state = None
        for s in range(9):
            if s == 1:
                vip.release()
                npool[0] = tc.alloc_tile_pool(name="norm", bufs=1)
            if s == 4:
                kip.release()
                qip.release()
                opool[0] = tc.alloc_tile_pool(name="outp", bufs=1)

            fl = list(fillers[s])
            n_emitted = 0
            a2h = [None]

            if s < 8:
                half, pair = s // 4, s % 4
                # pA single-buffered: the consuming AV step 2j of the next
                # slot runs at chunk j//2 <= j, before exp(j) needs the buf.
                # pB double-buffered: its AV steps run in the slot's 2nd half.
                ptA = [ppool.tile([128, SQH], dt.bfloat16, name=f"pA{j}", tag=f"pA{j}", bufs=1) for j in range(nkv)]
                ptB = [ppool.tile([128, SQH], dt.bfloat16, name=f"pB{j}", tag=f"pB{j}", bufs=2) for j in range(nkv)]
                cur_state = (ptA, ptB, pair, half)

                for j in range(nkv):
                    nc.tensor.ldweights(weights=wq_sb[0][:, 0:128])
                    # row-tiled score pair: head A rows 0-63 -> tile (0,0),
                    # head B rows 64-127 -> tile (64,0); concurrent in PE
                    scA = scp.tile([128, SQH], dt.float32, name="scA", tag="scA")
                    scB = scp.tile([128, SQH], dt.float32, name="scB", tag="scB")
                    for t in range(2):
                        nc.tensor.matmul(
                            scA[:, ts(t, 512)],
                            kh_sb[pair][0:64, ts(j, 128)],
                            qh_sb[half][pair][0:64, ts(t, 512)],
                            start=True, stop=True,
                        )
                        nc.tensor.matmul(
                            scB[:, ts(t, 512)],
                            kh_sb[pair][64:128, ts(j, 128)],
                            qh_sb[half][pair][64:128, ts(t, 512)],
                            start=True, stop=True,
                        )
                    nc.scalar.activation(
                        ptA[j][:], scA[:], mybir.ActivationFunctionType.Exp,
                        bias=mb_sb[:, j:j + 1], scale=SCALE,
                    )
                    nc.scalar.activation(
                        ptB[j][:], scB[:], mybir.ActivationFunctionType.Exp,
                        bias=mb_sb[:, j:j + 1], scale=SCALE,
                    )
                    if prev_state is not None:
                        av_step(prev_state, 2 * j, a2h)
                        av_step(prev_state, 2 * j + 1, a2h)
                    # spread filler units across the slot's chunks
                    want = (j + 1) * len(fl) // nkv
                    while n_emitted < want:
                        fl[n_emitted]()
                        n_emitted += 1
                prev_state = cur_state
            else:
                # drain slot: AV for slot 7, then out-proj H1.  PE warmers
                # (dependency-free matmuls) keep HAM at full clock through
                # the final norm-chain waits; out-proj units must be emitted
                # after the pair-3 norms (program order defines tile deps).
                for k in range(2 * nkv):
                    av_step(prev_state, k, a2h)
                    warm_mm()
                for i, f in enumerate(fl):
                    f()
                    if i < 6:
                        warm_mm()

        for pool in (opool[0], npool[0], avp, scp, prj, ppool, pp):
            pool.release()

    nc.compile()
    _PROGRAM_CACHE[skv] = nc
    return nc


def make_in_maps(q, k, v, mask, Wq, Wk, Wv, Wo, skv):
    """Host-side shard/compact/transpose/cast. Returns per-core input dicts."""
    in_maps = []
    valid = mask != 0
    for core in range(N_CORES):
        b, hg = core // 2, core % 2
        idx = np.nonzero(valid[b])[0]
        cnt = len(idx)

        kc = np.zeros((skv, E), np.float32)
        vc = np.zeros((skv, E), np.float32)
        kc[:cnt] = k[b][idx]
        vc[:cnt] = v[b][idx]

        mbias = np.zeros((skv,), np.float32)
        mbias[cnt:] = NEG
        # [128, nkv]: column j = kv chunk j
        mb2 = np.ascontiguousarray(mbias.reshape(-1, 128).T)

        rows = slice(hg * DHC, (hg + 1) * DHC)
        in_maps.append(dict(
            qT=np.ascontiguousarray(q[b].T).astype(BF16),
            kT=np.ascontiguousarray(kc.T).astype(BF16),
            vT=np.ascontiguousarray(vc.T).astype(BF16),
            wqT=np.ascontiguousarray(Wq[rows, :].T).astype(BF16),
            wkT=np.ascontiguousarray(Wk[rows, :].T).astype(BF16),
            wvT=np.ascontiguousarray(Wv[rows, :].T).astype(BF16),
            woT=np.ascontiguousarray(Wo[:, rows].T).astype(BF16),
            mb=mb2,
        ))
    return in_maps


def _numpy_fallback(q, k, v, mask, Wq, bq, Wk, bk, Wv, bv, Wo, bo):
    out = np.zeros((B, SQ, E), np.float32)
    for b in range(B):
        qh = (q[b] @ Wq.T + bq).reshape(SQ, H_TOT, D).transpose(1, 0, 2)
        kh = (k[b] @ Wk.T + bk).reshape(-1, H_TOT, D).transpose(1, 0, 2)
        vh = (v[b] @ Wv.T + bv).reshape(-1, H_TOT, D).transpose(1, 0, 2)
        att = np.einsum("hqd,hkd->hqk", qh, kh) * SCALE
        valid = mask[b] != 0
        if not valid.any():
            out[b] = bo
            continue
        att = np.where(valid[None, None, :], att, -np.inf)
        att = att - att.max(-1, keepdims=True)
        att = np.exp(att)
        att /= att.sum(-1, keepdims=True)
        o = np.einsum("hqk,hkd->hqd", att, vh)
        o = o.transpose(1, 0, 2).reshape(SQ, E)
        out[b] = o @ Wo.T + bo
    return out


def kernel(**inputs):
    global LAST_RESULTS
    q = np.asarray(inputs["q"], np.float32)
    k = np.asarray(inputs["k"], np.float32)
    v = np.asarray(inputs["v"], np.float32)
    mask = np.asarray(inputs["mask"])
    Wq, bq = np.asarray(inputs["Wq"], np.float32), np.asarray(inputs["bq"], np.float32)
    Wk, bk = np.asarray(inputs["Wk"], np.float32), np.asarray(inputs["bk"], np.float32)
    Wv, bv = np.asarray(inputs["Wv"], np.float32), np.asarray(inputs["bv"], np.float32)
    Wo, bo = np.asarray(inputs["Wo"], np.float32), np.asarray(inputs["bo"], np.float32)

    if any(np.abs(x).max() > 0 for x in (bq, bk, bv)):
        # q/k/v biases are zero in this problem's setup; a nonzero bias
        # would need the augmented-contraction path, so fall back.
        return _numpy_fallback(q, k, v, mask, Wq, bq, Wk, bk, Wv, bv, Wo, bo)

    valid = mask != 0
    counts = valid.sum(axis=1)
    if counts.max() == 0:
        return np.broadcast_to(bo, (B, SQ, E)).astype(np.float32).copy()

    skv = int(-(-counts.max() // 128) * 128)
    nc = build_program(skv)
    in_maps = make_in_maps(q, k, v, mask, Wq, Wk, Wv, Wo, skv)

    res = bass_utils.run_bass_kernel_spmd(nc, in_maps, core_ids=list(range(N_CORES)))
    LAST_RESULTS = res

    out = np.empty((B, SQ, E), np.float32)
    for b in range(B):
        if counts[b] == 0:
            out[b] = bo
        else:
            p0 = res.results[2 * b]["outT"]
            p1 = res.results[2 * b + 1]["outT"]
            out[b] = p0.T + p1.T + bo
    return out


# revision 30
# speedup vs baseline: 2.0839x; 1.0189x over previous
"""Multi-head attention (batched, key-padding mask) Trainium2 Bass kernel — v2.

Problem: nn_MultiHeadBatched
  q,k,v: [B=4, S=2048, E=1024] fp32; mask: [B, 2048] int32 (key padding)
  16 heads, head_dim 64; torch-Linear style q/k/v/out projections.

Sharding (8 cores): core c handles batch b=c//2 and head group hg=c%2
(8 heads each).  q/k/v projections are column-parallel over the head
group; out-projection is row-parallel — each core produces a partial
[E, Sq] output and the host sums the two partials per batch (+ bo).

v2 structure (single NeuronCore program, SPMD over 8 cores):
  - Host compacts the KV sequence per batch to the valid (mask!=0)
    positions, pads to a multiple of 128 (SKV); padded positions get an
    additive -1e30 bias folded into the ScalarE exp activation.
  - All matmuls bf16 with fp32 PSUM accumulation.
  - Scores computed transposed ([kv, q]); softmax normalizer Z comes
    from an all-ones column appended to each head's V (row 64 of the AV
    accumulation).  No max-subtraction (scores/8 ~ N(0,1)).
  - Slot pipeline over (q-half, head-pair): slot s = (half s//4, pair
    s%4) computes scores+exp for its half/pair while the PE also runs
    AV for slot s-1, interleaved chunk by chunk.  Head pairs live in
    PE row groups 0-63 / 64-127, so the two heads' score matmuls are
    issued back-to-back with tile_position (0,0)/(64,0) and execute
    CONCURRENTLY in the PE array (row tiling) — 2x score throughput.
  - The attention phase is ScalarE(exp)-bound; all projections (Q, K,
    V, out) are emitted as filler units inside the slots' chunk loops
    so the PE does them in exp-wait slack.  Q input is loaded and
    projected per q-half to cut SBUF pressure.
  - 1/Z: the Z row [1, 1024] is spread across 128 partitions via a
    SBUF->SBUF DMA, reciprocal'd as [128, 8] (DVE reciprocal is ~8
    cycles/elem — free-dim size is what costs), and DMA'd back.
"""

import os
import sys

import numpy as np

sys.path.insert(0, "/opt/trn_rl_repo")

import concourse.bass as bass
import concourse.bacc as bacc
import concourse.mybir as mybir
import concourse.tile as tile
from concourse import bass_utils

import ml_dtypes

BF16 = ml_dtypes.bfloat16

B, SQ, E = 4, 2048, 1024
H_TOT, D = 16, 64
HPC = H_TOT // 2            # heads per core (head-group split in 2)
DHC = HPC * D               # 512 projected channels per core
NE = E // 128               # contraction chunks
NDH = DHC // 128            # dh chunks per core
SQH = SQ // 2               # q-half width
NEG = -1.0e30
SCALE = D ** -0.5

N_CORES = 8

_PROGRAM_CACHE = {}
LAST_RESULTS = None


def _chunks512(n):
    out = []
    o = 0
    while o < n:
        w = min(512, n - o)
        out.append((o, w))
        o += w
    return out


def build_program(skv):
    """Build + compile the single-core SPMD Bass program for padded KV
    length `skv` (multiple of 128)."""
    if skv in _PROGRAM_CACHE:
        return _PROGRAM_CACHE[skv]

    nkv = skv // 128
    dt = mybir.dt

    nc = bacc.Bacc(
        "TRN2",
        target_bir_lowering=False,
        debug=False,
        enable_asserts=False,
        num_devices=N_CORES,
    )

    # DRAM I/O (per-core shapes)
    qT = nc.dram_tensor("qT", [E, SQ], dt.bfloat16, kind="ExternalInput").ap()
    kT = nc.dram_tensor("kT", [E, skv], dt.bfloat16, kind="ExternalInput").ap()
    vT = nc.dram_tensor("vT", [E, skv], dt.bfloat16, kind="ExternalInput").ap()
    wqT = nc.dram_tensor("wqT", [E, DHC], dt.bfloat16, kind="ExternalInput").ap()
    wkT = nc.dram_tensor("wkT", [E, DHC], dt.bfloat16, kind="ExternalInput").ap()
    wvT = nc.dram_tensor("wvT", [E, DHC], dt.bfloat16, kind="ExternalInput").ap()
    woT = nc.dram_tensor("woT", [DHC, E], dt.bfloat16, kind="ExternalInput").ap()
    mb = nc.dram_tensor("mb", [128, nkv], dt.float32, kind="ExternalInput").ap()
    # bf16 partials: halves the output DMA; host sums the two partials in
    # fp32 (+bo), adding only ~0.3% rel err against a 2% gate
    outT = nc.dram_tensor("outT", [E, SQ], dt.bfloat16, kind="ExternalOutput").ap()

    ts = bass.ts
    kvchunks = _chunks512(skv)

    with tile.TileContext(nc) as tc:
        pp = tc.alloc_tile_pool(name="persist", bufs=1)

        # Persistent SBUF tensors
        wq_sb = [pp.tile([128, DHC], dt.bfloat16, name=f"wq{e}", tag=f"wq{e}") for e in range(NE)]
        wk_sb = [pp.tile([128, DHC], dt.bfloat16, name=f"wk{e}", tag=f"wk{e}") for e in range(NE)]
        wv_sb = [pp.tile([128, DHC], dt.bfloat16, name=f"wv{e}", tag=f"wv{e}") for e in range(NE)]
        wo_sb = [pp.tile([128, E], dt.bfloat16, name=f"wo{c}", tag=f"wo{c}") for c in range(NDH)]
        # qh/aall split per q-half: separate tiles kill false WAR deps
        # between one half's reads and the other half's writes.
        qh_sb = [[pp.tile([128, SQH], dt.bfloat16, name=f"qh{h}_{c}", tag=f"qh{h}_{c}") for c in range(NDH)]
                 for h in range(2)]
        kh_sb = [pp.tile([128, skv], dt.bfloat16, name=f"kh{c}", tag=f"kh{c}") for c in range(NDH)]
        # V with per-head interleaved ones column: [kv, 8*(64+1)]
        va_sb = [pp.tile([128, HPC * (D + 1)], dt.bfloat16, name=f"va{j}", tag=f"va{j}") for j in range(nkv)]
        aall_sb = [[pp.tile([128, SQH], dt.bfloat16, name=f"aall{h}_{c}", tag=f"aall{h}_{c}") for c in range(NDH)]
                   for h in range(2)]
        mb_sb = pp.tile([128, nkv], dt.float32, name="mbt", tag="mbt")

        for j in range(nkv):
            nc.gpsimd.memset(va_sb[j][:, D::D + 1], 1.0)

        # Input pools (released as the projections complete; right-side
        # stack so mid-stream release doesn't violate LIFO pool order)
        qip = tc.alloc_tile_pool(name="qinp", bufs=1, side="right")
        kip = tc.alloc_tile_pool(name="kinp", bufs=1, side="right")
        vip = tc.alloc_tile_pool(name="vinp", bufs=1, side="right")

        # P tiles: per slot, 2 heads x nkv chunks of [128, SQH] bf16
        ppool = tc.alloc_tile_pool(name="ppool", bufs=2)

        # PSUM pools: prj 2 banks + scores 4 banks + AV 2 banks = 8
        prj = tc.alloc_tile_pool(name="prj", bufs=2, space="PSUM")
        scp = tc.alloc_tile_pool(name="scp", bufs=1, space="PSUM")
        avp = tc.alloc_tile_pool(name="avp", bufs=1, space="PSUM")

        npool = [None]  # allocated after vip release
        opool = [None]  # allocated after qip/kip release

        # ---------------- input DMAs ----------------
        q1_sb = [qip.tile([128, SQH], dt.bfloat16, name=f"q{e}", tag=f"q{e}") for e in range(NE)]
        k_sb = [kip.tile([128, skv], dt.bfloat16, name=f"k{e}", tag=f"k{e}") for e in range(NE)]
        v_sb = [vip.tile([128, skv], dt.bfloat16, name=f"v{e}", tag=f"v{e}") for e in range(NE)]
        # DMA order matches first-use order: K-proj(0) first, then exp's
        # mask bias, Q-proj(0, H0), then slot-0's V-proj fillers.
        for e in range(NE):
            nc.sync.dma_start(wk_sb[e][:], wkT[ts(e, 128), :])
            nc.sync.dma_start(k_sb[e][:], kT[ts(e, 128), :])
        nc.sync.dma_start(mb_sb[:], mb[:])
        for e in range(NE):
            nc.sync.dma_start(wq_sb[e][:], wqT[ts(e, 128), :])
            nc.sync.dma_start(q1_sb[e][:], qT[ts(e, 128), 0:SQH])
        for e in range(NE):
            nc.sync.dma_start(wv_sb[e][:], wvT[ts(e, 128), :])
            nc.sync.dma_start(v_sb[e][:], vT[ts(e, 128), :])
        for c in range(NDH):
            nc.sync.dma_start(wo_sb[c][:], woT[ts(c, 128), :])

        # ---------------- projection / out-proj unit emitters ----------------
        def _proj_ps(ptag):
            if ptag is None:
                return prj.tile([128, 512], dt.float32, name="pps", tag="prj")
            return scp.tile([128, SQH], dt.float32, name=ptag, tag=ptag)[:, 0:512]

        def kproj_unit(c, o, w, ptag=None):
            def emit():
                kps = _proj_ps(ptag)
                for e in range(NE):
                    nc.tensor.matmul(
                        kps[:, 0:w], wk_sb[e][:, ts(c, 128)], k_sb[e][:, o:o + w],
                        start=(e == 0), stop=(e == NE - 1),
                    )
                nc.vector.tensor_copy(kh_sb[c][:, o:o + w], kps[:, 0:w])
            return emit

        def qproj_unit(c, half, t, q_tiles, ptag=None):
            def emit():
                qps = _proj_ps(ptag)
                for e in range(NE):
                    nc.tensor.matmul(
                        qps[:], wq_sb[e][:, ts(c, 128)], q_tiles[e][:, ts(t, 512)],
                        start=(e == 0), stop=(e == NE - 1),
                    )
                nc.vector.tensor_copy(qh_sb[half][c][:, ts(t, 512)], qps[:])
            return emit

        def vproj_unit(j):
            def emit():
                vps = prj.tile([128, DHC], dt.float32, name="vps", tag="prj")
                for e in range(NE):
                    nc.tensor.matmul(
                        vps[:], v_sb[e][:, ts(j, 128)], wv_sb[e][:],
                        start=(e == 0), stop=(e == NE - 1),
                    )
                dst = va_sb[j].rearrange("p (h x) -> p h x", x=D + 1)[:, :, 0:D]
                src = vps.rearrange("p (h x) -> p h x", x=D)
                nc.vector.tensor_copy(dst, src)
            return emit

        q2_holder = {}

        def qdma2_unit():
            def emit():
                q2 = [qip.tile([128, SQH], dt.bfloat16, name=f"q{e}", tag=f"q{e}") for e in range(NE)]
                for e in range(NE):
                    nc.sync.dma_start(q2[e][:], qT[ts(e, 128), SQH:SQ])
                q2_holder["t"] = q2
            return emit

        def qproj2_unit(c, t):
            def emit():
                qps = prj.tile([128, 512], dt.float32, name="qps", tag="prj")
                for e in range(NE):
                    nc.tensor.matmul(
                        qps[:], wq_sb[e][:, ts(c, 128)], q2_holder["t"][e][:, ts(t, 512)],
                        start=(e == 0), stop=(e == NE - 1),
                    )
                nc.vector.tensor_copy(qh_sb[1][c][:, ts(t, 512)], qps[:])
            return emit

        def oproj_unit(half, eo, t):
            def emit():
                ops = prj.tile([128, 512], dt.float32, name="ops", tag="prj")
                for c in range(NDH):
                    nc.tensor.matmul(
                        ops[:], wo_sb[c][:, ts(eo, 128)],
                        aall_sb[half][c][:, ts(t, 512)],
                        start=(c == 0), stop=(c == NDH - 1),
                    )
                ob = opool[0].tile([128, 512], dt.bfloat16, name="ob", tag="ob", bufs=4)
                nc.vector.tensor_copy(ob[:], ops[:])
                nc.sync.dma_start(
                    outT[ts(eo, 128), half * SQH + t * 512:half * SQH + (t + 1) * 512], ob[:])
            return emit

        # ---------------- normalization ----------------
        ZW = SQH // 128  # 8

        def emit_norm(hp, half, a2):
            c, r = hp // 2, hp % 2
            np_ = npool[0]
            au = np_.tile([D + 1, SQH], dt.float32, name="au", tag="au", bufs=3)
            nc.vector.tensor_copy(au[:], a2[:])
            # spread Z across partitions; reciprocal cost is free-dim-size bound
            zt = np_.tile([128, ZW], dt.float32, name="zt", tag="zt", bufs=2)
            nc.sync.dma_start(zt[:], au[D:D + 1, :])
            rz8 = np_.tile([128, ZW], dt.float32, name="rz8", tag="rz8", bufs=2)
            nc.vector.reciprocal(rz8[:], zt[:])
            rzr = np_.tile([1, SQH], dt.float32, name="rzr", tag="rzr", bufs=2)
            nc.sync.dma_start(rzr[:], rz8[:])
            rb = np_.tile([D, SQH], dt.float32, name="rb", tag="rb", bufs=2)
            nc.gpsimd.partition_broadcast(rb[:], rzr[:])
            # mul on DVE, not gpsimd: keeping Pool broadcast-only avoids the
            # ~8us Q7 library reload between broadcast and multiply kernels.
            nc.vector.tensor_mul(
                aall_sb[half][c][r * 64:(r + 1) * 64, :],
                au[0:D, :], rb[:],
            )

        # ---------------- AV machinery ----------------
        def av_step(state, k, a2h):
            ptA, ptB, pair, half = state
            local = 0 if k < nkv else 1
            j = k - nkv * local
            pts = ptA if local == 0 else ptB
            hp = 2 * pair + local
            if j == 0:
                a2h[0] = avp.tile([D + 1, SQH], dt.float32, name="a2", tag="a2")
            a2 = a2h[0]
            for t in range(2):
                nc.tensor.matmul(
                    a2[:, ts(t, 512)],
                    va_sb[j][:, hp * (D + 1):(hp + 1) * (D + 1)],
                    pts[j][:, ts(t, 512)],
                    start=(j == 0), stop=(j == nkv - 1),
                )
            if j == nkv - 1:
                emit_norm(hp, half, a2)

        # ---------------- filler schedule ----------------
        fillers = {s: [] for s in range(9)}
        # slot 0: K(1) head chunk / Q(1,H0) early (needed at slot 1 scores),
        # V units interleaved (V chunk j needed by slot 1's AV at chunk j/2);
        # K(1) tail chunks and the last V units shift into slot 1's front.
        s0 = [vproj_unit(j) for j in range(max(nkv - 2, 1))]
        pri = [kproj_unit(1, kvchunks[0][0], kvchunks[0][1])]
        pri += [qproj_unit(1, 0, t, q1_sb) for t in range(2)]
        for i, u in enumerate(pri):
            s0.insert(2 * i + 1, u)
        fillers[0] = s0
        fillers[1] = [kproj_unit(1, o, w) for (o, w) in kvchunks[1:]]
        fillers[1] += [vproj_unit(j) for j in range(max(nkv - 2, 1), nkv)]
        for (o, w) in kvchunks:
            fillers[1].append(kproj_unit(2, o, w))
        fillers[1] += [qproj_unit(2, 0, t, q1_sb) for t in range(2)]
        for (o, w) in kvchunks:
            fillers[2].append(kproj_unit(3, o, w))
        fillers[2] += [qproj_unit(3, 0, t, q1_sb) for t in range(2)]
        fillers[2] += [qdma2_unit()]
        fillers[2] += [qproj2_unit(0, t) for t in range(2)]
        fillers[2] += [qproj2_unit(1, t) for t in range(2)]
        fillers[3] += [qproj2_unit(2, t) for t in range(2)]
        fillers[4] += [qproj2_unit(3, t) for t in range(2)]
        oh0 = [oproj_unit(0, eo, t) for eo in range(NE) for t in range(2)]
        fillers[5] = oh0[0:5]
        fillers[6] = oh0[5:10]
        fillers[7] = oh0[10:16]
        fillers[8] = [oproj_unit(1, eo, t) for eo in range(NE) for t in range(2)]

        def warm_mm():
            # dependency-free matmul into a dead PSUM tile: keeps the PE
            # HAM-busy through norm-chain waits so the tail runs at 2.4GHz
            wps = scp.tile([128, SQH], dt.float32, name="scA", tag="scA")
            nc.tensor.matmul(
                wps[:, 0:512], kh_sb[0][0:64, 0:128], qh_sb[1][0][0:64, 0:512],
                start=True, stop=True,
            )

        # ---------------- lead-in: K(0), Q(0, H0) ----------------
        # Q units get the scores' PSUM tags so none of the 5 lead units
        # serialize on prj-pool rotation
        for (o, w) in kvchunks:
            kproj_unit(0, o, w)()
        qproj_unit(0, 0, 0, q1_sb, ptag="scA")()
        qproj_unit(0, 0, 1, q1_sb, ptag="scB")()

        # ---------------- slot loop ----------------
        prev_state = None
        for s in range(9):
            if s == 1:
                vip.release()
                npool[0] = tc.alloc_tile_pool(name="norm", bufs=1)
            if s == 4:
                kip.release()
                qip.release()
                opool[0] = tc.alloc_tile_pool(name="outp", bufs=1)

            fl = list(fillers[s])
            n_emitted = 0
            a2h = [None]

            if s < 8:
                half, pair = s // 4, s % 4
                # pA single-buffered: the consuming AV step 2j of the next
                # slot runs at chunk j//2 <= j, before exp(j) needs the buf.
                # pB double-buffered: its AV steps run in the slot's 2nd half.
                ptA = [ppool.tile([128, SQH], dt.bfloat16, name=f"pA{j}", tag=f"pA{j}", bufs=1) for j in range(nkv)]
                ptB = [ppool.tile([128, SQH], dt.bfloat16, name=f"pB{j}", tag=f"pB{j}", bufs=2) for j in range(nkv)]
                cur_state = (ptA, ptB, pair, half)

                for j in range(nkv):
                    nc.tensor.ldweights(weights=wq_sb[0][:, 0:128])
                    # row-tiled score pair: head A rows 0-63 -> tile (0,0),
                    # head B rows 64-127 -> tile (64,0); concurrent in PE
                    scA = scp.tile([128, SQH], dt.float32, name="scA", tag="scA")
                    scB = scp.tile([128, SQH], dt.float32, name="scB", tag="scB")
                    for t in range(2):
                        nc.tensor.matmul(
                            scA[:, ts(t, 512)],
                            kh_sb[pair][0:64, ts(j, 128)],
                            qh_sb[half][pair][0:64, ts(t, 512)],
                            start=True, stop=True,
                        )
                        nc.tensor.matmul(
                            scB[:, ts(t, 512)],
                            kh_sb[pair][64:128, ts(j, 128)],
                            qh_sb[half][pair][64:128, ts(t, 512)],
                            start=True, stop=True,
                        )
                    nc.scalar.activation(
                        ptA[j][:], scA[:], mybir.ActivationFunctionType.Exp,
                        bias=mb_sb[:, j:j + 1], scale=SCALE,
                    )
                    nc.scalar.activation(
                        ptB[j][:], scB[:], mybir.ActivationFunctionType.Exp,
                        bias=mb_sb[:, j:j + 1], scale=SCALE,
                    )
                    if prev_state is not None:
                        av_step(prev_state, 2 * j, a2h)
                        av_step(prev_state, 2 * j + 1, a2h)
                    # spread filler units across the slot's chunks
                    want = (j + 1) * len(fl) // nkv
                    while n_emitted < want:
                        fl[n_emitted]()
                        n_emitted += 1
                prev_state = cur_state
            else:
                # drain slot: AV for slot 7, then out-proj H1.  PE warmers
                # (dependency-free matmuls) keep HAM at full clock through
                # the final norm-chain waits; out-proj units must be emitted
                # after the pair-3 norms (program order defines tile deps).
                for k in range(2 * nkv):
                    av_step(prev_state, k, a2h)
                    if k >= 2 * nkv - 5:
                        warm_mm()
                for i, f in enumerate(fl):
                    f()
                    if i < 6:
                        warm_mm()

        for pool in (opool[0], npool[0], avp, scp, prj, ppool, pp):
            pool.release()

    nc.compile()
    _PROGRAM_CACHE[skv] = nc
    return nc


def make_in_maps(q, k, v, mask, Wq, Wk, Wv, Wo, skv):
    """Host-side shard/compact/transpose/cast. Returns per-core input dicts."""
    in_maps = []
    valid = mask != 0
    for core in range(N_CORES):
        b, hg = core // 2, core % 2
        idx = np.nonzero(valid[b])[0]
        cnt = len(idx)

        kc = np.zeros((skv, E), np.float32)
        vc = np.zeros((skv, E), np.float32)
        kc[:cnt] = k[b][idx]
        vc[:cnt] = v[b][idx]

        mbias = np.zeros((skv,), np.float32)
        mbias[cnt:] = NEG
        # [128, nkv]: column j = kv chunk j
        mb2 = np.ascontiguousarray(mbias.reshape(-1, 128).T)

        rows = slice(hg * DHC, (hg + 1) * DHC)
        in_maps.append(dict(
            qT=np.ascontiguousarray(q[b].T).astype(BF16),
            kT=np.ascontiguousarray(kc.T).astype(BF16),
            vT=np.ascontiguousarray(vc.T).astype(BF16),
            wqT=np.ascontiguousarray(Wq[rows, :].T).astype(BF16),
            wkT=np.ascontiguousarray(Wk[rows, :].T).astype(BF16),
            wvT=np.ascontiguousarray(Wv[rows, :].T).astype(BF16),
            woT=np.ascontiguousarray(Wo[:, rows].T).astype(BF16),
            mb=mb2,
        ))
    return in_maps


def _numpy_fallback(q, k, v, mask, Wq, bq, Wk, bk, Wv, bv, Wo, bo):
    out = np.zeros((B, SQ, E), np.float32)
    for b in range(B):
        qh = (q[b] @ Wq.T + bq).reshape(SQ, H_TOT, D).transpose(1, 0, 2)
        kh = (k[b] @ Wk.T + bk).reshape(-1, H_TOT, D).transpose(1, 0, 2)
        vh = (v[b] @ Wv.T + bv).reshape(-1, H_TOT, D).transpose(1, 0, 2)
        att = np.einsum("hqd,hkd->hqk", qh, kh) * SCALE
        valid = mask[b] != 0
        if not valid.any():
            out[b] = bo
            continue
        att = np.where(valid[None, None, :], att, -np.inf)
        att = att - att.max(-1, keepdims=True)
        att = np.exp(att)
        att /= att.sum(-1, keepdims=True)
        o = np.einsum("hqk,hkd->hqd", att, vh)
        o = o.transpose(1, 0, 2).reshape(SQ, E)
        out[b] = o @ Wo.T + bo
    return out


def kernel(**inputs):
    global LAST_RESULTS
    q = np.asarray(inputs["q"], np.float32)
    k = np.asarray(inputs["k"], np.float32)
    v = np.asarray(inputs["v"], np.float32)
    mask = np.asarray(inputs["mask"])
    Wq, bq = np.asarray(inputs["Wq"], np.float32), np.asarray(inputs["bq"], np.float32)
    Wk, bk = np.asarray(inputs["Wk"], np.float32), np.asarray(inputs["bk"], np.float32)
    Wv, bv = np.asarray(inputs["Wv"], np.float32), np.asarray(inputs["bv"], np.float32)
    Wo, bo = np.asarray(inputs["Wo"], np.float32), np.asarray(inputs["bo"], np.float32)

    if any(np.abs(x).max() > 0 for x in (bq, bk, bv)):
        # q/k/v biases are zero in this problem's setup; a nonzero bias
        # would need the augmented-contraction path, so fall back.
        return _numpy_fallback(q, k, v, mask, Wq, bq, Wk, bk, Wv, bv, Wo, bo)

    valid = mask != 0
    counts = valid.sum(axis=1)
    if counts.max() == 0:
        return np.broadcast_to(bo, (B, SQ, E)).astype(np.float32).copy()

    skv = int(-(-counts.max() // 128) * 128)
    nc = build_program(skv)
    in_maps = make_in_maps(q, k, v, mask, Wq, Wk, Wv, Wo, skv)

    res = bass_utils.run_bass_kernel_spmd(nc, in_maps, core_ids=list(range(N_CORES)))
    LAST_RESULTS = res

    out = np.empty((B, SQ, E), np.float32)
    for b in range(B):
        if counts[b] == 0:
            out[b] = bo
        else:
            p0 = np.asarray(res.results[2 * b]["outT"], np.float32)
            p1 = np.asarray(res.results[2 * b + 1]["outT"], np.float32)
            out[b] = p0.T + p1.T + bo
    return out
